# revision 1
# baseline (speedup 1.0000x reference)
"""Multi-head attention (dense transformer block) on 8 Trainium2 NeuronCores.

Sharding: pure data-parallel over (batch=4) x (query half=2) -> 8 shards.
Each core computes, for its batch element b and query-token half:
  V  = x_b @ Wv.T         (natural layout, per-head 65-column interleave with
                           a trailing ones column for the softmax denominator)
  then per head-pair p (heads 2p, 2p+1), with projections interleaved into
  the attention stream:
    Qt_p = (Wq @ xq.T)[pair rows]   (transposed, 128 x 1024)
    Kt_p = (Wk @ x.T)[pair rows]    (transposed, 128 x 2048)
    per 128-key chunk: St for both heads lands in one 2-bank PSUM tile via a
      row-paired matmul pair, one wide exp(St/8) on ACT produces Pt, and one
      M=65 matmul per head accumulates [V.T @ Pt ; ones.T @ Pt] so the
      softmax numerator and denominator come from the same instruction.
    Ot = O_unnorm * (1/Z), with 1/Z broadcast down 64 partitions via tiny
      ones outer-product matmuls.
  out = Ot.T @ Wo.T + bo  (natural layout, written to DRAM)

K/V are computed redundantly by the two cores sharing a batch element; no
collectives are needed and every core writes a disjoint output slice.

Matmul operands are bf16 (fp32 PSUM accumulation); measured scale-relative
absmax error vs the fp32 reference is ~3e-3.
"""

import contextlib

import numpy as np
import ml_dtypes

import concourse.bass as bass
import concourse.tile as tile
import concourse.mybir as mybir
from concourse.bass_utils import run_bass_kernel_spmd

F32 = mybir.dt.float32
F32R = mybir.dt.float32r
BF16 = mybir.dt.bfloat16
EXP = mybir.ActivationFunctionType.Exp

D = 1024          # d_model
S = 2048          # sequence length
NH = 16           # heads
DH = 64           # head dim
QL = 1024         # query rows per core
NCORES = 8


def split_multi_waits(nc):
    """The walrus build in this container accepts at most one sync-wait per
    instruction; move extra waits onto same-engine nops inserted before the
    offending instruction."""
    k = 0
    for f in nc.m.functions:
        for bb in f.blocks:
            out, changed = [], False
            for inst in bb.instructions:
                si = inst.sync_info
                waits = list(si.on_wait) if si and si.on_wait else []
                if len(waits) > 1:
                    changed = True
                    for w in waits[:-1]:
                        nop = mybir.InstNoOp(name=f"wsplit-{k}", ins=[], outs=[])
                        k += 1
                        nop.engine = inst.engine
                        nop.sync_info = mybir.SyncInfo(on_wait=[w], on_update=[])
                        nc.register_instruction(nop, overwrite=True)
                        out.append(nop)
                    si.on_wait = waits[-1:]
                out.append(inst)
            if changed:
                bb.instructions = out


def build_program(repeat=1, knock=None):
    nc = bass.Bass()
    xqT = nc.declare_dram_parameter("xqT", [D, QL], BF16, isOutput=False)
    xT = nc.declare_dram_parameter("xT", [D, S], BF16, isOutput=False)
    wqT = nc.declare_dram_parameter("wqT", [D, D], BF16, isOutput=False)
    wkT = nc.declare_dram_parameter("wkT", [D, D], BF16, isOutput=False)
    wvT = nc.declare_dram_parameter("wvT", [D, D], BF16, isOutput=False)
    woT = nc.declare_dram_parameter("woT", [D, D], BF16, isOutput=False)
    bq2 = nc.declare_dram_parameter("bq2", [128, 8], F32, isOutput=False)
    bk2 = nc.declare_dram_parameter("bk2", [128, 8], F32, isOutput=False)
    bvb = nc.declare_dram_parameter("bvb", [128, D], F32, isOutput=False)
    bob = nc.declare_dram_parameter("bob", [128, D], F32, isOutput=False)
    ones2 = nc.declare_dram_parameter("ones2", [2, 64], F32R, isOutput=False)
    # bench-only: unique input signature per variant so stale NEFF caches
    # (keyed on HLO signature, not the embedded BIR) cannot serve a
    # previous program variant.
    tag = None
    if repeat > 1:
        tag = nc.declare_dram_parameter("tag", [1, repeat], F32, isOutput=False)
    out = nc.declare_dram_parameter("out", [QL, D], F32, isOutput=True)

    with tile.TileContext(nc) as tc:
        loop = tc.For_i(0, repeat, 1) if repeat > 1 else contextlib.nullcontext()
        with loop, \
             tc.tile_pool(name="persist", bufs=1) as pp, \
             tc.tile_pool(name="qk", bufs=2) as qkp, \
             tc.tile_pool(name="pt", bufs=3) as ptp, \
             tc.tile_pool(name="rz", bufs=2) as rzp:
            vg = [pp.tile([128, NH * (DH + 1)], BF16, name=f"vg{t}", tag=f"vg{t}")
                  for t in range(16)]
            ot = [pp.tile([128, QL], BF16, name=f"ot{p}", tag=f"ot{p}") for p in range(8)]
            bq_sb = pp.tile([128, 8], F32, name="bq_sb", tag="bq_sb")
            bk_sb = pp.tile([128, 8], F32, name="bk_sb", tag="bk_sb")
            bvb_sb = pp.tile([128, D], F32, name="bvb_sb", tag="bvb_sb")
            bob_sb = pp.tile([128, D], F32, name="bob_sb", tag="bob_sb")
            ones_sb = pp.tile([128, 64], F32R, name="ones_sb", tag="ones_sb")
            if tag is not None:
                tag_sb = pp.tile([1, repeat], F32, name="tag_sb", tag="tag_sb")
                nc.sync.dma_start(tag_sb[:], tag[:])
            # ones rows at partition 0 and partition 32 (matmul lhsT base must
            # match its rhs base; rz rows live at partitions 0 and 32)
            nc.sync.dma_start(ones_sb[0:1, :], ones2[0:1, :])
            nc.sync.dma_start(ones_sb[32:33, :], ones2[1:2, :])
            nc.sync.dma_start(bq_sb[:], bq2[:])
            nc.sync.dma_start(bk_sb[:], bk2[:])
            nc.sync.dma_start(bvb_sb[:], bvb[:])
            nc.sync.dma_start(bob_sb[:], bob[:])

            # resident activations and Q/K weights (bf16)
            xt_sb = [pp.tile([128, S], BF16, name=f"xt{d}", tag=f"xt{d}")
                     for d in range(8)]
            xq_sb = [pp.tile([128, QL], BF16, name=f"xq{d}", tag=f"xq{d}")
                     for d in range(8)]
            wq_sb = [pp.tile([128, D], BF16, name=f"wq{d}", tag=f"wq{d}")
                     for d in range(8)]
            wk_sb = [pp.tile([128, D], BF16, name=f"wk{d}", tag=f"wk{d}")
                     for d in range(8)]
            for d in range(8):
                nc.sync.dma_start(xt_sb[d][:], xT[128 * d:128 * (d + 1), :])
                nc.sync.dma_start(xq_sb[d][:], xqT[128 * d:128 * (d + 1), :])
                nc.sync.dma_start(wq_sb[d][:], wqT[128 * d:128 * (d + 1), :])
                nc.sync.dma_start(wk_sb[d][:], wkT[128 * d:128 * (d + 1), :])

            # ---- V projection (natural layout, interleaved ones columns).
            with tc.tile_pool(name="wv", bufs=1) as wvp, \
                 tc.tile_pool(name="psV", bufs=4, space="PSUM") as psvp:
                wv_sb = [wvp.tile([128, D], BF16, name=f"wv{d}", tag=f"wv{d}")
                         for d in range(8)]
                for d in range(8):
                    nc.sync.dma_start(wv_sb[d][:], wvT[128 * d:128 * (d + 1), :])
                for ti in range(16):
                    for hf in range(2):
                        ps = psvp.tile([128, 512], F32, name="psv", tag="psv", bufs=4)
                        for d in range(8):
                            nc.tensor.matmul(
                                ps[:], xt_sb[d][:, 128 * ti:128 * (ti + 1)],
                                wv_sb[d][:, 512 * hf:512 * (hf + 1)],
                                start=(d == 0), stop=(d == 7))
                        dst = vg[ti][:, 520 * hf:520 * (hf + 1)].rearrange(
                            "p (h w) -> p h w", w=65)[:, :, 0:64]
                        nc.vector.tensor_add(
                            dst,
                            ps[:].rearrange("p (h w) -> p h w", w=64),
                            bvb_sb[:, 512 * hf:512 * (hf + 1)].rearrange(
                                "p (h w) -> p h w", w=64))
                    nc.vector.memset(
                        vg[ti][:].rearrange("p (h w) -> p h w", w=65)[:, :, 64:65], 1.0)

            # ---- per head-pair: Q/K projection then attention.
            stp_cm = tc.tile_pool(name="psSt", bufs=2, space="PSUM")
            pop_cm = tc.tile_pool(name="psO", bufs=2, space="PSUM")
            pspp_cm = tc.tile_pool(name="psP", bufs=1, space="PSUM")
            psbp_cm = tc.tile_pool(name="psB", bufs=1, space="PSUM")
            stp = stp_cm.__enter__()
            pop = pop_cm.__enter__()
            pspp = pspp_cm.__enter__()
            psbp = psbp_cm.__enter__()
            if knock == "attn":
                for p in range(8):
                    nc.vector.memset(ot[p][:], 0.0)
            for p in (range(0) if knock == "attn" else range(8)):
                qt_p = qkp.tile([128, QL], BF16, name="qt_p", tag="qt", bufs=2)
                kt_p = qkp.tile([128, S], BF16, name="kt_p", tag="kt", bufs=2)
                for qb in range(2):
                    ps = pspp.tile([128, 512], F32, name="psq", tag="psp", bufs=1)
                    for d in range(8):
                        nc.tensor.matmul(
                            ps[:], wq_sb[d][:, 128 * p:128 * (p + 1)],
                            xq_sb[d][:, 512 * qb:512 * (qb + 1)],
                            start=(d == 0), stop=(d == 7))
                    nc.vector.tensor_scalar_add(
                        qt_p[:, 512 * qb:512 * (qb + 1)], ps[:], bq_sb[:, p:p + 1])
                for tb in range(4):
                    ps = pspp.tile([128, 512], F32, name="psk", tag="psp", bufs=1)
                    for d in range(8):
                        nc.tensor.matmul(
                            ps[:], wk_sb[d][:, 128 * p:128 * (p + 1)],
                            xt_sb[d][:, 512 * tb:512 * (tb + 1)],
                            start=(d == 0), stop=(d == 7))
                    nc.vector.tensor_scalar_add(
                        kt_p[:, 512 * tb:512 * (tb + 1)], ps[:], bk_sb[:, p:p + 1])

                c0 = 130 * p          # head 2p columns in vg
                c1 = 130 * p + 65     # head 2p+1 columns in vg
                for qb in range(2):
                    qs = slice(512 * qb, 512 * (qb + 1))
                    po0 = pop.tile([128, 512], F32, name="po0", tag="po", bufs=2)
                    po1 = pop.tile([128, 512], F32, name="po1", tag="po", bufs=2)
                    for k in range(16):
                        ks = slice(128 * k, 128 * (k + 1))
                        st = stp.tile([128, 1024], F32, name="st", tag="st", bufs=2)
                        nc.tensor.matmul(st[:, 0:512], kt_p[0:64, ks], qt_p[0:64, qs],
                                         start=True, stop=True)
                        nc.tensor.matmul(st[:, 512:1024], kt_p[64:128, ks],
                                         qt_p[64:128, qs], start=True, stop=True)
                        pt = ptp.tile([128, 1024], BF16, name="pt", tag="pt", bufs=3)
                        nc.scalar.activation(pt[:], st[:], EXP, scale=0.125)
                        first, last = (k == 0), (k == 15)
                        # fused numerator+denominator: lhsT = [V_head | ones]
                        nc.tensor.matmul(po0[0:65, :], vg[k][:, c0:c0 + 65],
                                         pt[:, 0:512], start=first, stop=last)
                        nc.tensor.matmul(po1[0:65, :], vg[k][:, c1:c1 + 65],
                                         pt[:, 512:1024], start=first, stop=last)
                    rz = rzp.tile([128, 512], F32R, name="rz", tag="rz", bufs=2)
                    with nc.allow_low_precision(reason="1/Z fed to f32r matmul"):
                        nc.vector.reciprocal(rz[0:1, :], po0[64:65, :])
                        nc.vector.reciprocal(rz[32:33, :], po1[64:65, :])
                    # broadcast 1/Z down 64 partitions: ones outer products on
                    # distinct row groups run concurrently.
                    pb0 = psbp.tile([128, 512], F32, name="pb0", tag="psb", bufs=1)
                    pb1 = psbp.tile([128, 512], F32, name="pb1", tag="psb", bufs=1)
                    nc.tensor.matmul(pb0[0:64, :], ones_sb[0:1, :], rz[0:1, :],
                                     start=True, stop=True)
                    nc.tensor.matmul(pb1[0:64, :], ones_sb[32:33, :], rz[32:33, :],
                                     start=True, stop=True)
                    rb0 = rzp.tile([128, 512], F32, name="rb0", tag="rb0", bufs=2)
                    rb1 = rzp.tile([128, 512], F32, name="rb1", tag="rb1", bufs=2)
                    nc.vector.tensor_copy(rb0[0:64, :], pb0[0:64, :])
                    nc.vector.tensor_copy(rb1[0:64, :], pb1[0:64, :])
                    nc.vector.tensor_mul(ot[p][0:64, qs], po0[0:64, :], rb0[0:64, :])
                    nc.vector.tensor_mul(ot[p][64:128, qs], po1[0:64, :], rb1[0:64, :])

            # ---- output projection + bias, natural layout.
            with tc.tile_pool(name="wo", bufs=1) as wop, \
                 tc.tile_pool(name="osb", bufs=3) as op_:
                wo_sb = [wop.tile([128, D], BF16, name=f"wo{d}", tag=f"wo{d}")
                         for d in range(8)]
                for d in range(8):
                    nc.sync.dma_start(wo_sb[d][:], woT[128 * d:128 * (d + 1), :])
                for t8 in range(8):
                    for hf in range(2):
                        ps = pspp.tile([128, 512], F32, name="pso", tag="psp", bufs=1)
                        for p in range(8):
                            nc.tensor.matmul(
                                ps[:], ot[p][:, 128 * t8:128 * (t8 + 1)],
                                wo_sb[p][:, 512 * hf:512 * (hf + 1)],
                                start=(p == 0), stop=(p == 7))
                        osb = op_.tile([128, 512], F32, name="osb", tag="osb", bufs=3)
                        nc.vector.tensor_add(osb[:], ps[:], bob_sb[:, 512 * hf:512 * (hf + 1)])
                        nc.sync.dma_start(
                            out[128 * t8:128 * (t8 + 1), 512 * hf:512 * (hf + 1)], osb[:])
            psbp_cm.__exit__(None, None, None)
            pspp_cm.__exit__(None, None, None)
            pop_cm.__exit__(None, None, None)
            stp_cm.__exit__(None, None, None)

    split_multi_waits(nc)
    return nc


_CACHED_NC = None


def get_program():
    global _CACHED_NC
    if _CACHED_NC is None:
        _CACHED_NC = build_program()
    return _CACHED_NC


def make_in_maps(x, Wq, bq, Wk, bk, Wv, bv, Wo, bo):
    x = np.asarray(x, np.float32)
    bf = ml_dtypes.bfloat16
    shared = {
        "wqT": np.ascontiguousarray(np.asarray(Wq, np.float32).T).astype(bf),
        "wkT": np.ascontiguousarray(np.asarray(Wk, np.float32).T).astype(bf),
        "wvT": np.ascontiguousarray(np.asarray(Wv, np.float32).T).astype(bf),
        "woT": np.ascontiguousarray(np.asarray(Wo, np.float32).T).astype(bf),
        "bq2": np.ascontiguousarray(np.asarray(bq, np.float32).reshape(8, 128).T),
        "bk2": np.ascontiguousarray(np.asarray(bk, np.float32).reshape(8, 128).T),
        "bvb": np.ascontiguousarray(np.tile(np.asarray(bv, np.float32), (128, 1))),
        "bob": np.ascontiguousarray(np.tile(np.asarray(bo, np.float32), (128, 1))),
        "ones2": np.ones((2, 64), np.float32),
    }
    in_maps = []
    for c in range(NCORES):
        b, half = c // 2, c % 2
        m = dict(shared)
        m["xT"] = np.ascontiguousarray(x[b].T).astype(bf)
        m["xqT"] = np.ascontiguousarray(x[b, half * QL:(half + 1) * QL].T).astype(bf)
        in_maps.append(m)
    return in_maps


def kernel(x, Wq, bq, Wk, bk, Wv, bv, Wo, bo):
    nc = get_program()
    in_maps = make_in_maps(x, Wq, bq, Wk, bk, Wv, bv, Wo, bo)
    res = run_bass_kernel_spmd(nc, in_maps, list(range(NCORES)))
    out = np.empty((4, S, D), np.float32)
    for c in range(NCORES):
        b, half = c // 2, c % 2
        out[b, half * QL:(half + 1) * QL, :] = res.results[c]["out"]
    return out



# revision 30
# speedup vs baseline: 1.0888x; 1.0888x over previous
"""Multi-head attention (dense transformer block) on 8 Trainium2 NeuronCores.

Sharding: (batch=4) x (head-group=2) -> 8 shards, tensor-parallel over heads.
Core c handles batch b = c//2 and heads [8*hg, 8*hg+8) with hg = c%2:
Q/K/V weights column-sharded (512 of 1024 output dims per core), Wo
row-sharded; the two row-parallel partial outputs per batch element are
summed host-side (plus the bo bias) during the unshard. No collectives.

Per core: V projection into a per-head 65-column interleave (trailing ones
column fuses the softmax denominator into the attn.V matmul); per head pair
p (0..3): Q/K projected transposed [128, 2048]; per 512-query block, scores
for both heads land in one 2-bank PSUM tile via a row-paired matmul pair,
one wide exp(St/8) on ACT, and one M=65 matmul per head accumulates
[V|ones].T @ Pt over the 16 key chunks.  The numerators are evacuated to
ot[p] unnormalized (bf16), the two Z rows staged at partitions 0/32 of a
zr tile, one DVE reciprocal per query block, 1/Z broadcast down 64
partitions with ones outer-product matmuls, and one in-place multiply pair
normalizes ot.  Finally out_partial = Ot.T @ WoShard.T (f32, no bias).

Matmul operands are bf16 (fp32 PSUM accumulation).
"""

import numpy as np
import ml_dtypes

import concourse.bass as bass
import concourse.tile as tile
import concourse.mybir as mybir
from concourse.bass_utils import run_bass_kernel_spmd

F32 = mybir.dt.float32
F32R = mybir.dt.float32r
BF16 = mybir.dt.bfloat16
EXP = mybir.ActivationFunctionType.Exp

D = 1024          # d_model
S = 2048          # sequence length (full batch element per core)
NH = 16           # heads total
NHC = 8           # heads per core
NP = 4            # head pairs per core
DH = 64           # head dim
DC = 512          # output dims per core (NHC * DH)
NCORES = 8
VARIANT = 12      # bump to bust the HLO-signature-keyed NEFF cache


def split_multi_waits(nc):
    """The walrus build in this container accepts at most one sync-wait per
    instruction; move extra waits onto same-engine nops inserted before the
    offending instruction."""
    k = 0
    for f in nc.m.functions:
        for bb in f.blocks:
            out, changed = [], False
            for inst in bb.instructions:
                si = inst.sync_info
                waits = list(si.on_wait) if si and si.on_wait else []
                if len(waits) > 1:
                    changed = True
                    for w in waits[:-1]:
                        nop = mybir.InstNoOp(name=f"wsplit-{k}", ins=[], outs=[])
                        k += 1
                        nop.engine = inst.engine
                        nop.sync_info = mybir.SyncInfo(on_wait=[w], on_update=[])
                        nc.register_instruction(nop, overwrite=True)
                        out.append(nop)
                    si.on_wait = waits[-1:]
                out.append(inst)
            if changed:
                bb.instructions = out


def build_program():
    nc = bass.Bass()
    xT = nc.declare_dram_parameter("xT", [D, S], BF16, isOutput=False)
    wkT = nc.declare_dram_parameter("wkT", [D, DC], BF16, isOutput=False)
    wqT = nc.declare_dram_parameter("wqT", [D, DC], BF16, isOutput=False)
    wvT = nc.declare_dram_parameter("wvT", [D, DC], BF16, isOutput=False)
    bk2 = nc.declare_dram_parameter("bk2", [128, NP], F32, isOutput=False)
    bq2 = nc.declare_dram_parameter("bq2", [128, NP], F32, isOutput=False)
    bvb = nc.declare_dram_parameter("bvb", [128, DC], F32, isOutput=False)
    ones4 = nc.declare_dram_parameter("ones4", [4, 64], F32R, isOutput=False)
    tag = nc.declare_dram_parameter("tag", [1, VARIANT], F32, isOutput=False)
    woT = nc.declare_dram_parameter("woT", [DC, D], BF16, isOutput=False)
    out = nc.declare_dram_parameter("out", [S, D], F32, isOutput=True)

    with tile.TileContext(nc) as tc:
        with tc.tile_pool(name="pp", bufs=1) as pp, \
             tc.tile_pool(name="qk", bufs=2) as qkp, \
             tc.tile_pool(name="pt", bufs=3) as ptp, \
             tc.tile_pool(name="zr", bufs=2) as zrp, \
             tc.tile_pool(name="psS", bufs=2, space="PSUM") as stp, \
             tc.tile_pool(name="psA", bufs=4, space="PSUM") as psa:
            bk_sb = pp.tile([128, 16], F32, name="bk_sb", tag="bk_sb")
            bq_sb = pp.tile([128, 16], F32, name="bq_sb", tag="bq_sb")
            bvb_sb = pp.tile([128, DC], F32, name="bvb_sb", tag="bvb_sb")
            ones_sb = pp.tile([128, 64], F32R, name="ones_sb", tag="ones_sb")
            tag_sb = pp.tile([1, VARIANT], F32, name="tag_sb", tag="tag_sb")
            nc.sync.dma_start(bk_sb[:, 0:NP], bk2[:])
            nc.sync.dma_start(bq_sb[:, 0:NP], bq2[:])
            nc.sync.dma_start(bvb_sb[:], bvb[:])
            nc.sync.dma_start(tag_sb[:], tag[:])
            for s in range(4):
                nc.sync.dma_start(ones_sb[32 * s:32 * s + 1, :], ones4[s:s + 1, :])
            xt_sb = [pp.tile([128, S], BF16, name=f"xt{d}", tag=f"xt{d}")
                     for d in range(8)]
            wk_sb = [pp.tile([128, DC], BF16, name=f"wk{d}", tag=f"wk{d}")
                     for d in range(8)]
            wq_sb = [pp.tile([128, DC], BF16, name=f"wq{d}", tag=f"wq{d}")
                     for d in range(8)]
            wv_sb = [pp.tile([128, DC], BF16, name=f"wv{d}", tag=f"wv{d}")
                     for d in range(8)]
            for d in range(8):
                nc.sync.dma_start(wk_sb[d][:], wkT[128 * d:128 * (d + 1), :])
            for c in range(4):
                cs = slice(512 * c, 512 * (c + 1))
                for d in range(8):
                    nc.sync.dma_start(xt_sb[d][:, cs], xT[128 * d:128 * (d + 1), cs])
            for d in range(8):
                nc.sync.dma_start(wq_sb[d][:], wqT[128 * d:128 * (d + 1), :])
            for d in range(8):
                nc.sync.dma_start(wv_sb[d][:], wvT[128 * d:128 * (d + 1), :])
            vg = [pp.tile([128, 520], BF16, name=f"vg{t}", tag=f"vg{t}")
                  for t in range(16)]
            ots = [pp.tile([128, S], BF16, name=f"ot{i}", tag=f"ot{i}")
                   for i in range(NP)]

            def qk_proj(p):
                kt_p = qkp.tile([128, S], BF16, name="kt_p", tag="kt", bufs=2)
                qt_p = qkp.tile([128, S], BF16, name="qt_p", tag="qt", bufs=2)
                for tb in range(4):
                    ts = slice(512 * tb, 512 * (tb + 1))
                    ps = psa.tile([128, 512], F32, name="psk", tag="psa", bufs=4)
                    for d in range(8):
                        nc.tensor.matmul(
                            ps[:], wk_sb[d][:, 128 * p:128 * (p + 1)],
                            xt_sb[d][:, ts], start=(d == 0), stop=(d == 7))
                    nc.vector.tensor_scalar_add(kt_p[:, ts], ps[:],
                                                bk_sb[:, p:p + 1])
                for tb in range(4):
                    ts = slice(512 * tb, 512 * (tb + 1))
                    ps = psa.tile([128, 512], F32, name="psq", tag="psa", bufs=4)
                    for d in range(8):
                        nc.tensor.matmul(
                            ps[:], wq_sb[d][:, 128 * p:128 * (p + 1)],
                            xt_sb[d][:, ts], start=(d == 0), stop=(d == 7))
                    nc.vector.tensor_scalar_add(qt_p[:, ts], ps[:],
                                                bq_sb[:, p:p + 1])
                return kt_p, qt_p

            kt_p, qt_p = qk_proj(0)
            for p in range(NP):
                if p > 0:
                    kt_p, qt_p = qk_proj(p)
                c0, c1 = 130 * p, 130 * p + 65
                for qb in range(4):
                    qs2 = slice(512 * qb, 512 * (qb + 1))
                    po0 = psa.tile([128, 512], F32, name="po0", tag="psa", bufs=4)
                    po1 = psa.tile([128, 512], F32, name="po1", tag="psa", bufs=4)
                    for k in range(16):
                        if p == 0 and qb == 0:
                            # fused V projection for key chunk k
                            ps = psa.tile([128, 512], F32, name="psv",
                                          tag="psa", bufs=4)
                            for d in range(8):
                                nc.tensor.matmul(
                                    ps[:], xt_sb[d][:, 128 * k:128 * (k + 1)],
                                    wv_sb[d][:], start=(d == 0), stop=(d == 7))
                            dst = vg[k][:].rearrange(
                                "p (h w) -> p h w", w=65)[:, :, 0:64]
                            nc.vector.tensor_add(
                                dst, ps[:].rearrange("p (h w) -> p h w", w=64),
                                bvb_sb[:].rearrange("p (h w) -> p h w", w=64))
                            nc.vector.memset(
                                vg[k][:].rearrange(
                                    "p (h w) -> p h w", w=65)[:, :, 64:65], 1.0)
                        ks = slice(128 * k, 128 * (k + 1))
                        st = stp.tile([128, 1024], F32, name="st", tag="st",
                                      bufs=2)
                        nc.tensor.matmul(st[:, 0:512], kt_p[0:64, ks],
                                         qt_p[0:64, qs2], start=True, stop=True)
                        nc.tensor.matmul(st[:, 512:1024], kt_p[64:128, ks],
                                         qt_p[64:128, qs2], start=True, stop=True)
                        pt = ptp.tile([128, 1024], BF16, name="pt", tag="pt",
                                      bufs=3)
                        nc.scalar.activation(pt[:], st[:], EXP, scale=0.125)
                        first, last = (k == 0), (k == 15)
                        nc.tensor.matmul(po0[0:65, :], vg[k][:, c0:c0 + 65],
                                         pt[:, 0:512], start=first, stop=last)
                        nc.tensor.matmul(po1[0:65, :], vg[k][:, c1:c1 + 65],
                                         pt[:, 512:1024], start=first, stop=last)
                    zr = zrp.tile([128, 512], F32, name="zrt", tag="zrt", bufs=2)
                    nc.vector.tensor_copy(zr[0:1, :], po0[64:65, :])
                    nc.vector.tensor_copy(zr[32:33, :], po1[64:65, :])
                    nc.vector.tensor_copy(ots[p][0:64, qs2], po0[0:64, :])
                    nc.vector.tensor_copy(ots[p][64:128, qs2], po1[0:64, :])
                    rz = zrp.tile([128, 512], F32R, name="rz", tag="rz", bufs=2)
                    with nc.allow_low_precision(reason="1/Z fed to f32r matmul"):
                        nc.vector.reciprocal(rz[0:33, :], zr[0:33, :])
                    pb0 = psa.tile([128, 512], F32, name="pb0", tag="psa", bufs=4)
                    pb1 = psa.tile([128, 512], F32, name="pb1", tag="psa", bufs=4)
                    nc.tensor.matmul(pb0[0:64, :], ones_sb[0:1, :], rz[0:1, :],
                                     start=True, stop=True)
                    nc.tensor.matmul(pb1[0:64, :], ones_sb[32:33, :],
                                     rz[32:33, :], start=True, stop=True)
                    nc.vector.tensor_mul(ots[p][0:64, qs2], ots[p][0:64, qs2],
                                         pb0[0:64, :])
                    nc.vector.tensor_mul(ots[p][64:128, qs2],
                                         ots[p][64:128, qs2], pb1[0:64, :])

            with tc.tile_pool(name="wo", bufs=1) as wop, \
                 tc.tile_pool(name="osb", bufs=3) as op_:
                wo_sb = [wop.tile([128, 1024], BF16, name=f"wo{d}", tag=f"wo{d}")
                         for d in range(4)]
                for d in range(4):
                    nc.sync.dma_start(wo_sb[d][:], woT[128 * d:128 * (d + 1), :])
                for t8 in range(16):
                    for hf in range(2):
                        ps = psa.tile([128, 512], F32, name="pso", tag="psa",
                                      bufs=4)
                        for p in range(4):
                            nc.tensor.matmul(
                                ps[:], ots[p][:, 128 * t8:128 * (t8 + 1)],
                                wo_sb[p][:, 512 * hf:512 * (hf + 1)],
                                start=(p == 0), stop=(p == 3))
                        osb = op_.tile([128, 512], F32, name="osb", tag="osb",
                                       bufs=3)
                        nc.vector.tensor_copy(osb[:], ps[:])
                        nc.sync.dma_start(
                            out[128 * t8:128 * (t8 + 1),
                                512 * hf:512 * (hf + 1)], osb[:])

    split_multi_waits(nc)
    return nc


_CACHED_NC = None


def get_program():
    global _CACHED_NC
    if _CACHED_NC is None:
        _CACHED_NC = build_program()
    return _CACHED_NC


def make_in_maps(x, Wq, bq, Wk, bk, Wv, bv, Wo, bo):
    x = np.asarray(x, np.float32)
    bf = ml_dtypes.bfloat16
    WqT = np.ascontiguousarray(np.asarray(Wq, np.float32).T)
    WkT = np.ascontiguousarray(np.asarray(Wk, np.float32).T)
    WvT = np.ascontiguousarray(np.asarray(Wv, np.float32).T)
    WoT = np.ascontiguousarray(np.asarray(Wo, np.float32).T)
    bq = np.asarray(bq, np.float32)
    bk = np.asarray(bk, np.float32)
    bv = np.asarray(bv, np.float32)
    shard = []
    for hg in range(2):
        ds = slice(DC * hg, DC * (hg + 1))
        shard.append({
            "wqT": np.ascontiguousarray(WqT[:, ds]).astype(bf),
            "wkT": np.ascontiguousarray(WkT[:, ds]).astype(bf),
            "wvT": np.ascontiguousarray(WvT[:, ds]).astype(bf),
            "woT": np.ascontiguousarray(WoT[ds, :]).astype(bf),
            "bq2": np.ascontiguousarray(bq[ds].reshape(NP, 128).T),
            "bk2": np.ascontiguousarray(bk[ds].reshape(NP, 128).T),
            "bvb": np.ascontiguousarray(np.tile(bv[ds], (128, 1))),
            "ones4": np.ones((4, 64), np.float32),
            "tag": np.zeros((1, VARIANT), np.float32),
        })
    in_maps = []
    for c in range(NCORES):
        b, hg = c // 2, c % 2
        m = dict(shard[hg])
        m["xT"] = np.ascontiguousarray(x[b].T).astype(bf)
        in_maps.append(m)
    return in_maps


def assemble(results, bo):
    """Unshard: sum the two row-parallel partials per batch element + bias."""
    bo = np.asarray(bo, np.float32)
    out = np.empty((4, S, D), np.float32)
    for b in range(4):
        out[b] = results[2 * b]["out"] + results[2 * b + 1]["out"] + bo
    return out


def kernel(x, Wq, bq, Wk, bk, Wv, bv, Wo, bo):
    nc = get_program()
    in_maps = make_in_maps(x, Wq, bq, Wk, bk, Wv, bv, Wo, bo)
    res = run_bass_kernel_spmd(nc, in_maps, list(range(NCORES)))
    return assemble(res.results, bo)


# revision 33
# speedup vs baseline: 1.2145x; 1.1155x over previous
"""Multi-head attention (dense transformer block) on 8 Trainium2 NeuronCores.

Sharding: (batch=4) x (head-group=2) -> 8 shards, tensor-parallel over heads.
Core c handles batch b = c//2 and heads [8*hg, 8*hg+8) with hg = c%2:
Q/K/V weights column-sharded (512 of 1024 output dims per core), Wo
row-sharded; the two row-parallel partial outputs per batch element are
summed host-side (plus the bo bias) during the unshard. No collectives.

Per core: V projection into a per-head 65-column interleave (trailing ones
column fuses the softmax denominator into the attn.V matmul); per head pair
p (0..3): Q/K projected transposed [128, 2048]; per 512-query block, scores
for both heads land in one 2-bank PSUM tile via a row-paired matmul pair,
one wide exp(St/8) on ACT, and one M=65 matmul per head accumulates
[V|ones].T @ Pt over the 16 key chunks.  The numerators are evacuated to
ot[p] unnormalized (bf16), the two Z rows staged at partitions 0/32 of a
zr tile, one DVE reciprocal per query block, 1/Z broadcast down 64
partitions with ones outer-product matmuls, and one in-place multiply pair
normalizes ot.  Finally out_partial = Ot.T @ WoShard.T (f32, no bias).

Matmul operands are bf16 (fp32 PSUM accumulation).
"""

import numpy as np
import ml_dtypes

import concourse.bass as bass
import concourse.tile as tile
import concourse.mybir as mybir
from concourse.bass_utils import run_bass_kernel_spmd

F32 = mybir.dt.float32
F32R = mybir.dt.float32r
BF16 = mybir.dt.bfloat16
EXP = mybir.ActivationFunctionType.Exp

D = 1024          # d_model
S = 2048          # sequence length (full batch element per core)
NH = 16           # heads total
NHC = 8           # heads per core
NP = 4            # head pairs per core
DH = 64           # head dim
DC = 512          # output dims per core (NHC * DH)
NCORES = 8
VARIANT = 13      # bump to bust the HLO-signature-keyed NEFF cache


def split_multi_waits(nc):
    """The walrus build in this container accepts at most one sync-wait per
    instruction; move extra waits onto same-engine nops inserted before the
    offending instruction."""
    k = 0
    for f in nc.m.functions:
        for bb in f.blocks:
            out, changed = [], False
            for inst in bb.instructions:
                si = inst.sync_info
                waits = list(si.on_wait) if si and si.on_wait else []
                if len(waits) > 1:
                    changed = True
                    for w in waits[:-1]:
                        nop = mybir.InstNoOp(name=f"wsplit-{k}", ins=[], outs=[])
                        k += 1
                        nop.engine = inst.engine
                        nop.sync_info = mybir.SyncInfo(on_wait=[w], on_update=[])
                        nc.register_instruction(nop, overwrite=True)
                        out.append(nop)
                    si.on_wait = waits[-1:]
                out.append(inst)
            if changed:
                bb.instructions = out


def build_program():
    nc = bass.Bass()
    xT = nc.declare_dram_parameter("xT", [D, S], BF16, isOutput=False)
    wkT = nc.declare_dram_parameter("wkT", [D, DC], BF16, isOutput=False)
    wqT = nc.declare_dram_parameter("wqT", [D, DC], BF16, isOutput=False)
    wvT = nc.declare_dram_parameter("wvT", [D, DC], BF16, isOutput=False)
    bk2 = nc.declare_dram_parameter("bk2", [128, NP], F32, isOutput=False)
    bq2 = nc.declare_dram_parameter("bq2", [128, NP], F32, isOutput=False)
    bvb = nc.declare_dram_parameter("bvb", [128, DC], F32, isOutput=False)
    ones4 = nc.declare_dram_parameter("ones4", [4, 64], F32R, isOutput=False)
    tag = nc.declare_dram_parameter("tag", [1, VARIANT], F32, isOutput=False)
    woT = nc.declare_dram_parameter("woT", [DC, D], BF16, isOutput=False)
    out = nc.declare_dram_parameter("out", [S, D], F32, isOutput=True)

    with tile.TileContext(nc) as tc:
        with tc.tile_pool(name="pp", bufs=1) as pp, \
             tc.tile_pool(name="qk", bufs=2) as qkp, \
             tc.tile_pool(name="pt", bufs=3) as ptp, \
             tc.tile_pool(name="zr", bufs=2) as zrp, \
             tc.tile_pool(name="psS", bufs=2, space="PSUM") as stp, \
             tc.tile_pool(name="psA", bufs=4, space="PSUM") as psa:
            bk_sb = pp.tile([128, 16], F32, name="bk_sb", tag="bk_sb")
            bq_sb = pp.tile([128, 16], F32, name="bq_sb", tag="bq_sb")
            bvb_sb = pp.tile([128, DC], F32, name="bvb_sb", tag="bvb_sb")
            ones_sb = pp.tile([128, 64], F32R, name="ones_sb", tag="ones_sb")
            tag_sb = pp.tile([1, VARIANT], F32, name="tag_sb", tag="tag_sb")
            nc.sync.dma_start(bk_sb[:, 0:NP], bk2[:])
            nc.sync.dma_start(bq_sb[:, 0:NP], bq2[:])
            nc.sync.dma_start(bvb_sb[:], bvb[:])
            nc.sync.dma_start(tag_sb[:], tag[:])
            for s in range(4):
                nc.sync.dma_start(ones_sb[32 * s:32 * s + 1, :], ones4[s:s + 1, :])
            xt_sb = [pp.tile([128, S], BF16, name=f"xt{d}", tag=f"xt{d}")
                     for d in range(8)]
            wk_sb = [pp.tile([128, DC], BF16, name=f"wk{d}", tag=f"wk{d}")
                     for d in range(8)]
            wq_sb = [pp.tile([128, DC], BF16, name=f"wq{d}", tag=f"wq{d}")
                     for d in range(8)]
            wv_sb = [pp.tile([128, DC], BF16, name=f"wv{d}", tag=f"wv{d}")
                     for d in range(8)]
            for d in range(8):
                nc.sync.dma_start(wk_sb[d][:], wkT[128 * d:128 * (d + 1), :])
            for c in range(4):
                cs = slice(512 * c, 512 * (c + 1))
                for d in range(8):
                    nc.sync.dma_start(xt_sb[d][:, cs], xT[128 * d:128 * (d + 1), cs])
            for d in range(8):
                nc.sync.dma_start(wq_sb[d][:], wqT[128 * d:128 * (d + 1), :])
            for d in range(8):
                nc.sync.dma_start(wv_sb[d][:], wvT[128 * d:128 * (d + 1), :])
            vg = [pp.tile([128, 520], BF16, name=f"vg{t}", tag=f"vg{t}")
                  for t in range(16)]
            ots = [pp.tile([128, S], BF16, name=f"ot{i}", tag=f"ot{i}")
                   for i in range(NP)]

            def qk_proj(p):
                kt_p = qkp.tile([128, S], BF16, name="kt_p", tag="kt", bufs=2)
                qt_p = qkp.tile([128, S], BF16, name="qt_p", tag="qt", bufs=2)
                for tb in range(4):
                    ts = slice(512 * tb, 512 * (tb + 1))
                    ps = psa.tile([128, 512], F32, name="psk", tag="psa", bufs=4)
                    for d in range(8):
                        nc.tensor.matmul(
                            ps[:], wk_sb[d][:, 128 * p:128 * (p + 1)],
                            xt_sb[d][:, ts], start=(d == 0), stop=(d == 7))
                    nc.vector.tensor_scalar_add(kt_p[:, ts], ps[:],
                                                bk_sb[:, p:p + 1])
                for tb in range(4):
                    ts = slice(512 * tb, 512 * (tb + 1))
                    ps = psa.tile([128, 512], F32, name="psq", tag="psa", bufs=4)
                    for d in range(8):
                        nc.tensor.matmul(
                            ps[:], wq_sb[d][:, 128 * p:128 * (p + 1)],
                            xt_sb[d][:, ts], start=(d == 0), stop=(d == 7))
                    nc.vector.tensor_scalar_add(qt_p[:, ts], ps[:],
                                                bq_sb[:, p:p + 1])
                return kt_p, qt_p

            def make_tail(p, qs2, zr):
                """1/Z normalization for one finished query block.  Emitted
                one block LATE so the PE's in-order stream never stalls on
                the broadcast matmuls waiting for the DVE reciprocal."""
                def tail():
                    rz = zrp.tile([128, 512], F32R, name="rz", tag="rz", bufs=2)
                    with nc.allow_low_precision(reason="1/Z to f32r matmul"):
                        nc.vector.reciprocal(rz[0:33, :], zr[0:33, :])
                    pb0 = psa.tile([128, 512], F32, name="pb0", tag="psa", bufs=4)
                    pb1 = psa.tile([128, 512], F32, name="pb1", tag="psa", bufs=4)
                    nc.tensor.matmul(pb0[0:64, :], ones_sb[0:1, :], rz[0:1, :],
                                     start=True, stop=True)
                    nc.tensor.matmul(pb1[0:64, :], ones_sb[32:33, :],
                                     rz[32:33, :], start=True, stop=True)
                    nc.vector.tensor_mul(ots[p][0:64, qs2], ots[p][0:64, qs2],
                                         pb0[0:64, :])
                    nc.vector.tensor_mul(ots[p][64:128, qs2],
                                         ots[p][64:128, qs2], pb1[0:64, :])
                return tail

            pending = None
            kt_p, qt_p = qk_proj(0)
            for p in range(NP):
                if p > 0:
                    kt_p, qt_p = qk_proj(p)
                c0, c1 = 130 * p, 130 * p + 65
                for qb in range(4):
                    qs2 = slice(512 * qb, 512 * (qb + 1))
                    po0 = psa.tile([128, 512], F32, name="po0", tag="psa", bufs=4)
                    po1 = psa.tile([128, 512], F32, name="po1", tag="psa", bufs=4)
                    for k in range(16):
                        if p == 0 and qb == 0:
                            # fused V projection for key chunk k
                            ps = psa.tile([128, 512], F32, name="psv",
                                          tag="psa", bufs=4)
                            for d in range(8):
                                nc.tensor.matmul(
                                    ps[:], xt_sb[d][:, 128 * k:128 * (k + 1)],
                                    wv_sb[d][:], start=(d == 0), stop=(d == 7))
                            dst = vg[k][:].rearrange(
                                "p (h w) -> p h w", w=65)[:, :, 0:64]
                            nc.vector.tensor_add(
                                dst, ps[:].rearrange("p (h w) -> p h w", w=64),
                                bvb_sb[:].rearrange("p (h w) -> p h w", w=64))
                            nc.vector.memset(
                                vg[k][:].rearrange(
                                    "p (h w) -> p h w", w=65)[:, :, 64:65], 1.0)
                        ks = slice(128 * k, 128 * (k + 1))
                        st = stp.tile([128, 1024], F32, name="st", tag="st",
                                      bufs=2)
                        nc.tensor.matmul(st[:, 0:512], kt_p[0:64, ks],
                                         qt_p[0:64, qs2], start=True, stop=True)
                        nc.tensor.matmul(st[:, 512:1024], kt_p[64:128, ks],
                                         qt_p[64:128, qs2], start=True, stop=True)
                        pt = ptp.tile([128, 1024], BF16, name="pt", tag="pt",
                                      bufs=3)
                        nc.scalar.activation(pt[:], st[:], EXP, scale=0.125)
                        first, last = (k == 0), (k == 15)
                        nc.tensor.matmul(po0[0:65, :], vg[k][:, c0:c0 + 65],
                                         pt[:, 0:512], start=first, stop=last)
                        nc.tensor.matmul(po1[0:65, :], vg[k][:, c1:c1 + 65],
                                         pt[:, 512:1024], start=first, stop=last)
                    zr = zrp.tile([128, 512], F32, name="zrt", tag="zrt", bufs=2)
                    nc.vector.tensor_copy(zr[0:1, :], po0[64:65, :])
                    nc.vector.tensor_copy(zr[32:33, :], po1[64:65, :])
                    nc.vector.tensor_copy(ots[p][0:64, qs2], po0[0:64, :])
                    nc.vector.tensor_copy(ots[p][64:128, qs2], po1[0:64, :])
                    if pending is not None:
                        pending()
                    pending = make_tail(p, qs2, zr)
            pending()

            with tc.tile_pool(name="wo", bufs=1) as wop, \
                 tc.tile_pool(name="osb", bufs=3) as op_:
                wo_sb = [wop.tile([128, 1024], BF16, name=f"wo{d}", tag=f"wo{d}")
                         for d in range(4)]
                for d in range(4):
                    nc.sync.dma_start(wo_sb[d][:], woT[128 * d:128 * (d + 1), :])
                for t8 in range(16):
                    for hf in range(2):
                        ps = psa.tile([128, 512], F32, name="pso", tag="psa",
                                      bufs=4)
                        for p in range(4):
                            nc.tensor.matmul(
                                ps[:], ots[p][:, 128 * t8:128 * (t8 + 1)],
                                wo_sb[p][:, 512 * hf:512 * (hf + 1)],
                                start=(p == 0), stop=(p == 3))
                        osb = op_.tile([128, 512], F32, name="osb", tag="osb",
                                       bufs=3)
                        nc.vector.tensor_copy(osb[:], ps[:])
                        nc.sync.dma_start(
                            out[128 * t8:128 * (t8 + 1),
                                512 * hf:512 * (hf + 1)], osb[:])

    split_multi_waits(nc)
    return nc


_CACHED_NC = None


def get_program():
    global _CACHED_NC
    if _CACHED_NC is None:
        _CACHED_NC = build_program()
    return _CACHED_NC


def make_in_maps(x, Wq, bq, Wk, bk, Wv, bv, Wo, bo):
    x = np.asarray(x, np.float32)
    bf = ml_dtypes.bfloat16
    WqT = np.ascontiguousarray(np.asarray(Wq, np.float32).T)
    WkT = np.ascontiguousarray(np.asarray(Wk, np.float32).T)
    WvT = np.ascontiguousarray(np.asarray(Wv, np.float32).T)
    WoT = np.ascontiguousarray(np.asarray(Wo, np.float32).T)
    bq = np.asarray(bq, np.float32)
    bk = np.asarray(bk, np.float32)
    bv = np.asarray(bv, np.float32)
    shard = []
    for hg in range(2):
        ds = slice(DC * hg, DC * (hg + 1))
        shard.append({
            "wqT": np.ascontiguousarray(WqT[:, ds]).astype(bf),
            "wkT": np.ascontiguousarray(WkT[:, ds]).astype(bf),
            "wvT": np.ascontiguousarray(WvT[:, ds]).astype(bf),
            "woT": np.ascontiguousarray(WoT[ds, :]).astype(bf),
            "bq2": np.ascontiguousarray(bq[ds].reshape(NP, 128).T),
            "bk2": np.ascontiguousarray(bk[ds].reshape(NP, 128).T),
            "bvb": np.ascontiguousarray(np.tile(bv[ds], (128, 1))),
            "ones4": np.ones((4, 64), np.float32),
            "tag": np.zeros((1, VARIANT), np.float32),
        })
    in_maps = []
    for c in range(NCORES):
        b, hg = c // 2, c % 2
        m = dict(shard[hg])
        m["xT"] = np.ascontiguousarray(x[b].T).astype(bf)
        in_maps.append(m)
    return in_maps


def assemble(results, bo):
    """Unshard: sum the two row-parallel partials per batch element + bias."""
    bo = np.asarray(bo, np.float32)
    out = np.empty((4, S, D), np.float32)
    for b in range(4):
        out[b] = results[2 * b]["out"] + results[2 * b + 1]["out"] + bo
    return out


def kernel(x, Wq, bq, Wk, bk, Wv, bv, Wo, bo):
    nc = get_program()
    in_maps = make_in_maps(x, Wq, bq, Wk, bk, Wv, bv, Wo, bo)
    res = run_bass_kernel_spmd(nc, in_maps, list(range(NCORES)))
    return assemble(res.results, bo)


# revision 35
# speedup vs baseline: 1.2385x; 1.0197x over previous
"""Multi-head attention (dense transformer block) on 8 Trainium2 NeuronCores.

Sharding: (batch=4) x (head-group=2) -> 8 shards, tensor-parallel over heads.
Core c handles batch b = c//2 and heads [8*hg, 8*hg+8) with hg = c%2:
Q/K/V weights column-sharded (512 of 1024 output dims per core), Wo
row-sharded; the two row-parallel partial outputs per batch element are
summed host-side (plus the bo bias) during the unshard. No collectives.

Per core: V projection into a per-head 65-column interleave (trailing ones
column fuses the softmax denominator into the attn.V matmul); per head pair
p (0..3): Q/K projected transposed [128, 2048]; per 512-query block, scores
for both heads land in one 2-bank PSUM tile via a row-paired matmul pair,
one wide exp(St/8) on ACT, and one M=65 matmul per head accumulates
[V|ones].T @ Pt over the 16 key chunks.  The numerators are evacuated to
ot[p] unnormalized (bf16), the two Z rows staged at partitions 0/32 of a
zr tile, one DVE reciprocal per query block, 1/Z broadcast down 64
partitions with ones outer-product matmuls, and one in-place multiply pair
normalizes ot.  Finally out_partial = Ot.T @ WoShard.T (f32, no bias).

Matmul operands are bf16 (fp32 PSUM accumulation).
"""

import numpy as np
import ml_dtypes

import concourse.bass as bass
import concourse.tile as tile
import concourse.mybir as mybir
from concourse.bass_utils import run_bass_kernel_spmd

F32 = mybir.dt.float32
F32R = mybir.dt.float32r
BF16 = mybir.dt.bfloat16
EXP = mybir.ActivationFunctionType.Exp

D = 1024          # d_model
S = 2048          # sequence length (full batch element per core)
NH = 16           # heads total
NHC = 8           # heads per core
NP = 4            # head pairs per core
DH = 64           # head dim
DC = 512          # output dims per core (NHC * DH)
NCORES = 8
VARIANT = 14      # bump to bust the HLO-signature-keyed NEFF cache


def split_multi_waits(nc):
    """The walrus build in this container accepts at most one sync-wait per
    instruction; move extra waits onto same-engine nops inserted before the
    offending instruction."""
    k = 0
    for f in nc.m.functions:
        for bb in f.blocks:
            out, changed = [], False
            for inst in bb.instructions:
                si = inst.sync_info
                waits = list(si.on_wait) if si and si.on_wait else []
                if len(waits) > 1:
                    changed = True
                    for w in waits[:-1]:
                        nop = mybir.InstNoOp(name=f"wsplit-{k}", ins=[], outs=[])
                        k += 1
                        nop.engine = inst.engine
                        nop.sync_info = mybir.SyncInfo(on_wait=[w], on_update=[])
                        nc.register_instruction(nop, overwrite=True)
                        out.append(nop)
                    si.on_wait = waits[-1:]
                out.append(inst)
            if changed:
                bb.instructions = out


def build_program():
    nc = bass.Bass()
    xT = nc.declare_dram_parameter("xT", [D, S], BF16, isOutput=False)
    wkT = nc.declare_dram_parameter("wkT", [D, DC], BF16, isOutput=False)
    wqT = nc.declare_dram_parameter("wqT", [D, DC], BF16, isOutput=False)
    wvT = nc.declare_dram_parameter("wvT", [D, DC], BF16, isOutput=False)
    bk2 = nc.declare_dram_parameter("bk2", [128, NP], F32, isOutput=False)
    bq2 = nc.declare_dram_parameter("bq2", [128, NP], F32, isOutput=False)
    bvb = nc.declare_dram_parameter("bvb", [128, DC], F32, isOutput=False)
    ones4 = nc.declare_dram_parameter("ones4", [4, 64], F32R, isOutput=False)
    tag = nc.declare_dram_parameter("tag", [1, VARIANT], F32, isOutput=False)
    woT = nc.declare_dram_parameter("woT", [DC, D], BF16, isOutput=False)
    out = nc.declare_dram_parameter("out", [S, D], F32, isOutput=True)

    with tile.TileContext(nc) as tc:
        with tc.tile_pool(name="pp", bufs=1) as pp, \
             tc.tile_pool(name="qk", bufs=2) as qkp, \
             tc.tile_pool(name="pt", bufs=3) as ptp, \
             tc.tile_pool(name="zr", bufs=2) as zrp, \
             tc.tile_pool(name="psS", bufs=2, space="PSUM") as stp, \
             tc.tile_pool(name="psA", bufs=4, space="PSUM") as psa:
            bk_sb = pp.tile([128, 16], F32, name="bk_sb", tag="bk_sb")
            bq_sb = pp.tile([128, 16], F32, name="bq_sb", tag="bq_sb")
            bvb_sb = pp.tile([128, DC], F32, name="bvb_sb", tag="bvb_sb")
            ones_sb = pp.tile([128, 64], F32R, name="ones_sb", tag="ones_sb")
            tag_sb = pp.tile([1, VARIANT], F32, name="tag_sb", tag="tag_sb")
            nc.sync.dma_start(bk_sb[:, 0:NP], bk2[:])
            nc.sync.dma_start(bq_sb[:, 0:NP], bq2[:])
            nc.sync.dma_start(bvb_sb[:], bvb[:])
            nc.sync.dma_start(tag_sb[:], tag[:])
            for s in range(4):
                nc.sync.dma_start(ones_sb[32 * s:32 * s + 1, :], ones4[s:s + 1, :])
            xt_sb = [pp.tile([128, S], BF16, name=f"xt{d}", tag=f"xt{d}")
                     for d in range(8)]
            wk_sb = [pp.tile([128, DC], BF16, name=f"wk{d}", tag=f"wk{d}")
                     for d in range(8)]
            wq_sb = [pp.tile([128, DC], BF16, name=f"wq{d}", tag=f"wq{d}")
                     for d in range(8)]
            wv_sb = [pp.tile([128, DC], BF16, name=f"wv{d}", tag=f"wv{d}")
                     for d in range(8)]
            for d in range(8):
                nc.sync.dma_start(wk_sb[d][:], wkT[128 * d:128 * (d + 1), :])
            for c in range(4):
                cs = slice(512 * c, 512 * (c + 1))
                for d in range(8):
                    nc.sync.dma_start(xt_sb[d][:, cs], xT[128 * d:128 * (d + 1), cs])
            for d in range(8):
                nc.sync.dma_start(wq_sb[d][:], wqT[128 * d:128 * (d + 1), :])
            for d in range(8):
                nc.sync.dma_start(wv_sb[d][:], wvT[128 * d:128 * (d + 1), :])
            vg = [pp.tile([128, 520], BF16, name=f"vg{t}", tag=f"vg{t}")
                  for t in range(16)]
            ots = [pp.tile([128, S], BF16, name=f"ot{i}", tag=f"ot{i}")
                   for i in range(NP)]

            def qk_proj(p):
                kt_p = qkp.tile([128, S], BF16, name="kt_p", tag="kt", bufs=2)
                qt_p = qkp.tile([128, S], BF16, name="qt_p", tag="qt", bufs=2)
                for tb in range(4):
                    ts = slice(512 * tb, 512 * (tb + 1))
                    ps = psa.tile([128, 512], F32, name="psk", tag="psa", bufs=4)
                    for d in range(8):
                        nc.tensor.matmul(
                            ps[:], wk_sb[d][:, 128 * p:128 * (p + 1)],
                            xt_sb[d][:, ts], start=(d == 0), stop=(d == 7))
                    nc.vector.tensor_scalar_add(kt_p[:, ts], ps[:],
                                                bk_sb[:, p:p + 1])
                for tb in range(4):
                    ts = slice(512 * tb, 512 * (tb + 1))
                    ps = psa.tile([128, 512], F32, name="psq", tag="psa", bufs=4)
                    for d in range(8):
                        nc.tensor.matmul(
                            ps[:], wq_sb[d][:, 128 * p:128 * (p + 1)],
                            xt_sb[d][:, ts], start=(d == 0), stop=(d == 7))
                    nc.vector.tensor_scalar_add(qt_p[:, ts], ps[:],
                                                bq_sb[:, p:p + 1])
                return kt_p, qt_p

            def make_tail(p, qs2, zr):
                """1/Z normalization for one finished query block.  Emitted
                one block LATE so the PE's in-order stream never stalls on
                the broadcast matmuls waiting for the DVE reciprocal."""
                def tail():
                    rz = zrp.tile([128, 512], F32R, name="rz", tag="rz", bufs=2)
                    with nc.allow_low_precision(reason="1/Z to f32r matmul"):
                        nc.vector.reciprocal(rz[0:33, :], zr[0:33, :])
                    pb0 = psa.tile([128, 512], F32, name="pb0", tag="psa", bufs=4)
                    pb1 = psa.tile([128, 512], F32, name="pb1", tag="psa", bufs=4)
                    nc.tensor.matmul(pb0[0:64, :], ones_sb[0:1, :], rz[0:1, :],
                                     start=True, stop=True)
                    nc.tensor.matmul(pb1[0:64, :], ones_sb[32:33, :],
                                     rz[32:33, :], start=True, stop=True)
                    nc.vector.tensor_mul(ots[p][0:64, qs2], ots[p][0:64, qs2],
                                         pb0[0:64, :])
                    nc.vector.tensor_mul(ots[p][64:128, qs2],
                                         ots[p][64:128, qs2], pb1[0:64, :])
                return tail

            pending = None
            kt_p, qt_p = qk_proj(0)
            for p in range(NP):
                if p > 0:
                    kt_p, qt_p = qk_proj(p)
                c0, c1 = 130 * p, 130 * p + 65
                for qb in range(4):
                    qs2 = slice(512 * qb, 512 * (qb + 1))
                    po0 = psa.tile([128, 512], F32, name="po0", tag="psa", bufs=4)
                    po1 = psa.tile([128, 512], F32, name="po1", tag="psa", bufs=4)
                    for k in range(16):
                        if p == 0 and qb == 0:
                            # fused V projection for key chunk k
                            ps = psa.tile([128, 512], F32, name="psv",
                                          tag="psa", bufs=4)
                            for d in range(8):
                                nc.tensor.matmul(
                                    ps[:], xt_sb[d][:, 128 * k:128 * (k + 1)],
                                    wv_sb[d][:], start=(d == 0), stop=(d == 7))
                            dst = vg[k][:].rearrange(
                                "p (h w) -> p h w", w=65)[:, :, 0:64]
                            nc.vector.tensor_add(
                                dst, ps[:].rearrange("p (h w) -> p h w", w=64),
                                bvb_sb[:].rearrange("p (h w) -> p h w", w=64))
                            nc.vector.memset(
                                vg[k][:].rearrange(
                                    "p (h w) -> p h w", w=65)[:, :, 64:65], 1.0)
                        ks = slice(128 * k, 128 * (k + 1))
                        st = stp.tile([128, 1024], F32, name="st", tag="st",
                                      bufs=2)
                        nc.tensor.matmul(st[:, 0:512], kt_p[0:64, ks],
                                         qt_p[0:64, qs2], start=True, stop=True)
                        nc.tensor.matmul(st[:, 512:1024], kt_p[64:128, ks],
                                         qt_p[64:128, qs2], start=True, stop=True)
                        pt = ptp.tile([128, 1024], BF16, name="pt", tag="pt",
                                      bufs=3)
                        nc.scalar.activation(pt[:], st[:], EXP, scale=0.125)
                        first, last = (k == 0), (k == 15)
                        nc.tensor.matmul(po0[0:65, :], vg[k][:, c0:c0 + 65],
                                         pt[:, 0:512], start=first, stop=last)
                        nc.tensor.matmul(po1[0:65, :], vg[k][:, c1:c1 + 65],
                                         pt[:, 512:1024], start=first, stop=last)
                    zr = zrp.tile([128, 512], F32, name="zrt", tag="zrt", bufs=2)
                    nc.vector.tensor_copy(zr[0:1, :], po0[64:65, :])
                    nc.vector.tensor_copy(zr[32:33, :], po1[64:65, :])
                    # evacuate numerators on the (idle) scalar engine so the
                    # po slots free while DVE handles the Z rows
                    nc.scalar.copy(ots[p][0:64, qs2], po0[0:64, :])
                    nc.scalar.copy(ots[p][64:128, qs2], po1[0:64, :])
                    if pending is not None:
                        pending()
                    pending = make_tail(p, qs2, zr)
            pending()

            with tc.tile_pool(name="wo", bufs=1) as wop, \
                 tc.tile_pool(name="osb", bufs=3) as op_:
                wo_sb = [wop.tile([128, 1024], BF16, name=f"wo{d}", tag=f"wo{d}")
                         for d in range(4)]
                for d in range(4):
                    nc.sync.dma_start(wo_sb[d][:], woT[128 * d:128 * (d + 1), :])
                for t8 in range(16):
                    for hf in range(2):
                        ps = psa.tile([128, 512], F32, name="pso", tag="psa",
                                      bufs=4)
                        for p in range(4):
                            nc.tensor.matmul(
                                ps[:], ots[p][:, 128 * t8:128 * (t8 + 1)],
                                wo_sb[p][:, 512 * hf:512 * (hf + 1)],
                                start=(p == 0), stop=(p == 3))
                        osb = op_.tile([128, 512], F32, name="osb", tag="osb",
                                       bufs=3)
                        nc.vector.tensor_copy(osb[:], ps[:])
                        nc.sync.dma_start(
                            out[128 * t8:128 * (t8 + 1),
                                512 * hf:512 * (hf + 1)], osb[:])

    split_multi_waits(nc)
    return nc


_CACHED_NC = None


def get_program():
    global _CACHED_NC
    if _CACHED_NC is None:
        _CACHED_NC = build_program()
    return _CACHED_NC


def make_in_maps(x, Wq, bq, Wk, bk, Wv, bv, Wo, bo):
    x = np.asarray(x, np.float32)
    bf = ml_dtypes.bfloat16
    WqT = np.ascontiguousarray(np.asarray(Wq, np.float32).T)
    WkT = np.ascontiguousarray(np.asarray(Wk, np.float32).T)
    WvT = np.ascontiguousarray(np.asarray(Wv, np.float32).T)
    WoT = np.ascontiguousarray(np.asarray(Wo, np.float32).T)
    bq = np.asarray(bq, np.float32)
    bk = np.asarray(bk, np.float32)
    bv = np.asarray(bv, np.float32)
    shard = []
    for hg in range(2):
        ds = slice(DC * hg, DC * (hg + 1))
        shard.append({
            "wqT": np.ascontiguousarray(WqT[:, ds]).astype(bf),
            "wkT": np.ascontiguousarray(WkT[:, ds]).astype(bf),
            "wvT": np.ascontiguousarray(WvT[:, ds]).astype(bf),
            "woT": np.ascontiguousarray(WoT[ds, :]).astype(bf),
            "bq2": np.ascontiguousarray(bq[ds].reshape(NP, 128).T),
            "bk2": np.ascontiguousarray(bk[ds].reshape(NP, 128).T),
            "bvb": np.ascontiguousarray(np.tile(bv[ds], (128, 1))),
            "ones4": np.ones((4, 64), np.float32),
            "tag": np.zeros((1, VARIANT), np.float32),
        })
    in_maps = []
    for c in range(NCORES):
        b, hg = c // 2, c % 2
        m = dict(shard[hg])
        m["xT"] = np.ascontiguousarray(x[b].T).astype(bf)
        in_maps.append(m)
    return in_maps


def assemble(results, bo):
    """Unshard: sum the two row-parallel partials per batch element + bias."""
    bo = np.asarray(bo, np.float32)
    out = np.empty((4, S, D), np.float32)
    for b in range(4):
        out[b] = results[2 * b]["out"] + results[2 * b + 1]["out"] + bo
    return out


def kernel(x, Wq, bq, Wk, bk, Wv, bv, Wo, bo):
    nc = get_program()
    in_maps = make_in_maps(x, Wq, bq, Wk, bk, Wv, bv, Wo, bo)
    res = run_bass_kernel_spmd(nc, in_maps, list(range(NCORES)))
    return assemble(res.results, bo)


# revision 38
# speedup vs baseline: 1.2562x; 1.0143x over previous
"""Multi-head attention (dense transformer block) on 8 Trainium2 NeuronCores.

Sharding: (batch=4) x (head-group=2) -> 8 shards, tensor-parallel over heads.
Core c handles batch b = c//2 and heads [8*hg, 8*hg+8) with hg = c%2:
Q/K/V weights column-sharded (512 of 1024 output dims per core), Wo
row-sharded; the two row-parallel partial outputs per batch element are
summed host-side (plus the bo bias) during the unshard. No collectives.

Per core: V projection into a per-head 65-column interleave (trailing ones
column fuses the softmax denominator into the attn.V matmul); per head pair
p (0..3): Q/K projected transposed [128, 2048]; per 512-query block, scores
for both heads land in one 2-bank PSUM tile via a row-paired matmul pair,
one wide exp(St/8) on ACT, and one M=65 matmul per head accumulates
[V|ones].T @ Pt over the 16 key chunks.  The numerators are evacuated to
ot[p] unnormalized (bf16), the two Z rows staged at partitions 0/32 of a
zr tile, one DVE reciprocal per query block, 1/Z broadcast down 64
partitions with ones outer-product matmuls, and one in-place multiply pair
normalizes ot.  Finally out_partial = Ot.T @ WoShard.T (f32, no bias).

Matmul operands are bf16 (fp32 PSUM accumulation).
"""

import numpy as np
import ml_dtypes

import concourse.bass as bass
import concourse.tile as tile
import concourse.mybir as mybir
from concourse.bass_utils import run_bass_kernel_spmd

F32 = mybir.dt.float32
F32R = mybir.dt.float32r
BF16 = mybir.dt.bfloat16
EXP = mybir.ActivationFunctionType.Exp

D = 1024          # d_model
S = 2048          # sequence length (full batch element per core)
NH = 16           # heads total
NHC = 8           # heads per core
NP = 4            # head pairs per core
DH = 64           # head dim
DC = 512          # output dims per core (NHC * DH)
NCORES = 8
VARIANT = 15      # bump to bust the HLO-signature-keyed NEFF cache


def split_multi_waits(nc):
    """The walrus build in this container accepts at most one sync-wait per
    instruction; move extra waits onto same-engine nops inserted before the
    offending instruction."""
    k = 0
    for f in nc.m.functions:
        for bb in f.blocks:
            out, changed = [], False
            for inst in bb.instructions:
                si = inst.sync_info
                waits = list(si.on_wait) if si and si.on_wait else []
                if len(waits) > 1:
                    changed = True
                    for w in waits[:-1]:
                        nop = mybir.InstNoOp(name=f"wsplit-{k}", ins=[], outs=[])
                        k += 1
                        nop.engine = inst.engine
                        nop.sync_info = mybir.SyncInfo(on_wait=[w], on_update=[])
                        nc.register_instruction(nop, overwrite=True)
                        out.append(nop)
                    si.on_wait = waits[-1:]
                out.append(inst)
            if changed:
                bb.instructions = out


def build_program():
    nc = bass.Bass()
    xT = nc.declare_dram_parameter("xT", [D, S], BF16, isOutput=False)
    wkT = nc.declare_dram_parameter("wkT", [D, DC], BF16, isOutput=False)
    wqT = nc.declare_dram_parameter("wqT", [D, DC], BF16, isOutput=False)
    wvT = nc.declare_dram_parameter("wvT", [D, DC], BF16, isOutput=False)
    bk2 = nc.declare_dram_parameter("bk2", [128, NP], F32, isOutput=False)
    bq2 = nc.declare_dram_parameter("bq2", [128, NP], F32, isOutput=False)
    bvb = nc.declare_dram_parameter("bvb", [128, DC], F32, isOutput=False)
    ones4 = nc.declare_dram_parameter("ones4", [4, 64], F32R, isOutput=False)
    tag = nc.declare_dram_parameter("tag", [1, VARIANT], F32, isOutput=False)
    woT = nc.declare_dram_parameter("woT", [DC, D], BF16, isOutput=False)
    out = nc.declare_dram_parameter("out", [S, D], F32, isOutput=True)

    with tile.TileContext(nc) as tc:
        with tc.tile_pool(name="pp", bufs=1) as pp, \
             tc.tile_pool(name="qk", bufs=2) as qkp, \
             tc.tile_pool(name="pt", bufs=3) as ptp, \
             tc.tile_pool(name="zr", bufs=2) as zrp, \
             tc.tile_pool(name="psS", bufs=2, space="PSUM") as stp, \
             tc.tile_pool(name="psA", bufs=4, space="PSUM") as psa:
            bk_sb = pp.tile([128, 16], F32, name="bk_sb", tag="bk_sb")
            bq_sb = pp.tile([128, 16], F32, name="bq_sb", tag="bq_sb")
            bvb_sb = pp.tile([128, DC], F32, name="bvb_sb", tag="bvb_sb")
            ones_sb = pp.tile([128, 64], F32R, name="ones_sb", tag="ones_sb")
            tag_sb = pp.tile([1, VARIANT], F32, name="tag_sb", tag="tag_sb")
            nc.sync.dma_start(bk_sb[:, 0:NP], bk2[:])
            nc.sync.dma_start(bq_sb[:, 0:NP], bq2[:])
            nc.sync.dma_start(bvb_sb[:], bvb[:])
            nc.sync.dma_start(tag_sb[:], tag[:])
            for s in range(4):
                nc.sync.dma_start(ones_sb[32 * s:32 * s + 1, :], ones4[s:s + 1, :])
            xt_sb = [pp.tile([128, S], BF16, name=f"xt{d}", tag=f"xt{d}")
                     for d in range(8)]
            wk_sb = [pp.tile([128, DC], BF16, name=f"wk{d}", tag=f"wk{d}")
                     for d in range(8)]
            wq_sb = [pp.tile([128, DC], BF16, name=f"wq{d}", tag=f"wq{d}")
                     for d in range(8)]
            wv_sb = [pp.tile([128, DC], BF16, name=f"wv{d}", tag=f"wv{d}")
                     for d in range(8)]
            for d in range(8):
                nc.sync.dma_start(wk_sb[d][:], wkT[128 * d:128 * (d + 1), :])
            for d in range(8):
                nc.sync.dma_start(wq_sb[d][:], wqT[128 * d:128 * (d + 1), :])
            for c in range(4):
                cs = slice(512 * c, 512 * (c + 1))
                for d in range(8):
                    nc.sync.dma_start(xt_sb[d][:, cs], xT[128 * d:128 * (d + 1), cs])
            for d in range(8):
                nc.sync.dma_start(wv_sb[d][:], wvT[128 * d:128 * (d + 1), :])
            vg = [pp.tile([128, 520], BF16, name=f"vg{t}", tag=f"vg{t}")
                  for t in range(16)]
            ots = [pp.tile([128, S], BF16, name=f"ot{i}", tag=f"ot{i}")
                   for i in range(NP)]

            def qk_proj(p):
                kt_p = qkp.tile([128, S], BF16, name="kt_p", tag="kt", bufs=2)
                qt_p = qkp.tile([128, S], BF16, name="qt_p", tag="qt", bufs=2)
                # interleave K and Q per token block so each xt chunk feeds
                # both projections as soon as it lands (halves time-to-first-
                # score at kernel start)
                for tb in range(4):
                    ts = slice(512 * tb, 512 * (tb + 1))
                    ps = psa.tile([128, 512], F32, name="psk", tag="psa", bufs=4)
                    for d in range(8):
                        nc.tensor.matmul(
                            ps[:], wk_sb[d][:, 128 * p:128 * (p + 1)],
                            xt_sb[d][:, ts], start=(d == 0), stop=(d == 7))
                    nc.vector.tensor_scalar_add(kt_p[:, ts], ps[:],
                                                bk_sb[:, p:p + 1])
                    ps = psa.tile([128, 512], F32, name="psq", tag="psa", bufs=4)
                    for d in range(8):
                        nc.tensor.matmul(
                            ps[:], wq_sb[d][:, 128 * p:128 * (p + 1)],
                            xt_sb[d][:, ts], start=(d == 0), stop=(d == 7))
                    nc.vector.tensor_scalar_add(qt_p[:, ts], ps[:],
                                                bq_sb[:, p:p + 1])
                return kt_p, qt_p

            def make_tail(p, qs2, zr):
                """1/Z normalization for one finished query block.  Emitted
                one block LATE so the PE's in-order stream never stalls on
                the broadcast matmuls waiting for the DVE reciprocal."""
                def tail():
                    rz = zrp.tile([128, 512], F32R, name="rz", tag="rz", bufs=2)
                    with nc.allow_low_precision(reason="1/Z to f32r matmul"):
                        nc.vector.reciprocal(rz[0:33, :], zr[0:33, :])
                    pb0 = psa.tile([128, 512], F32, name="pb0", tag="psa", bufs=4)
                    pb1 = psa.tile([128, 512], F32, name="pb1", tag="psa", bufs=4)
                    nc.tensor.matmul(pb0[0:64, :], ones_sb[0:1, :], rz[0:1, :],
                                     start=True, stop=True)
                    nc.tensor.matmul(pb1[0:64, :], ones_sb[32:33, :],
                                     rz[32:33, :], start=True, stop=True)
                    nc.vector.tensor_mul(ots[p][0:64, qs2], ots[p][0:64, qs2],
                                         pb0[0:64, :])
                    nc.vector.tensor_mul(ots[p][64:128, qs2],
                                         ots[p][64:128, qs2], pb1[0:64, :])
                return tail

            pending = None
            kt_p, qt_p = qk_proj(0)
            for p in range(NP):
                if p > 0:
                    kt_p, qt_p = qk_proj(p)
                c0, c1 = 130 * p, 130 * p + 65
                for qb in range(4):
                    qs2 = slice(512 * qb, 512 * (qb + 1))
                    po0 = psa.tile([128, 512], F32, name="po0", tag="psa", bufs=4)
                    po1 = psa.tile([128, 512], F32, name="po1", tag="psa", bufs=4)
                    for k in range(16):
                        if p == 0 and qb == 0:
                            # fused V projection for key chunk k
                            ps = psa.tile([128, 512], F32, name="psv",
                                          tag="psa", bufs=4)
                            for d in range(8):
                                nc.tensor.matmul(
                                    ps[:], xt_sb[d][:, 128 * k:128 * (k + 1)],
                                    wv_sb[d][:], start=(d == 0), stop=(d == 7))
                            dst = vg[k][:].rearrange(
                                "p (h w) -> p h w", w=65)[:, :, 0:64]
                            nc.vector.tensor_add(
                                dst, ps[:].rearrange("p (h w) -> p h w", w=64),
                                bvb_sb[:].rearrange("p (h w) -> p h w", w=64))
                            nc.vector.memset(
                                vg[k][:].rearrange(
                                    "p (h w) -> p h w", w=65)[:, :, 64:65], 1.0)
                        ks = slice(128 * k, 128 * (k + 1))
                        st = stp.tile([128, 1024], F32, name="st", tag="st",
                                      bufs=2)
                        nc.tensor.matmul(st[:, 0:512], kt_p[0:64, ks],
                                         qt_p[0:64, qs2], start=True, stop=True)
                        nc.tensor.matmul(st[:, 512:1024], kt_p[64:128, ks],
                                         qt_p[64:128, qs2], start=True, stop=True)
                        pt = ptp.tile([128, 1024], BF16, name="pt", tag="pt",
                                      bufs=3)
                        nc.scalar.activation(pt[:], st[:], EXP, scale=0.125)
                        first, last = (k == 0), (k == 15)
                        nc.tensor.matmul(po0[0:65, :], vg[k][:, c0:c0 + 65],
                                         pt[:, 0:512], start=first, stop=last)
                        nc.tensor.matmul(po1[0:65, :], vg[k][:, c1:c1 + 65],
                                         pt[:, 512:1024], start=first, stop=last)
                    zr = zrp.tile([128, 512], F32, name="zrt", tag="zrt", bufs=2)
                    nc.vector.tensor_copy(zr[0:1, :], po0[64:65, :])
                    nc.vector.tensor_copy(zr[32:33, :], po1[64:65, :])
                    # evacuate numerators on the (idle) scalar engine so the
                    # po slots free while DVE handles the Z rows
                    nc.scalar.copy(ots[p][0:64, qs2], po0[0:64, :])
                    nc.scalar.copy(ots[p][64:128, qs2], po1[0:64, :])
                    if pending is not None:
                        pending()
                    pending = make_tail(p, qs2, zr)
            pending()

            with tc.tile_pool(name="wo", bufs=1) as wop, \
                 tc.tile_pool(name="osb", bufs=3) as op_:
                wo_sb = [wop.tile([128, 1024], BF16, name=f"wo{d}", tag=f"wo{d}")
                         for d in range(4)]
                for d in range(4):
                    nc.sync.dma_start(wo_sb[d][:], woT[128 * d:128 * (d + 1), :])
                for t8 in range(16):
                    for hf in range(2):
                        ps = psa.tile([128, 512], F32, name="pso", tag="psa",
                                      bufs=4)
                        for p in range(4):
                            nc.tensor.matmul(
                                ps[:], ots[p][:, 128 * t8:128 * (t8 + 1)],
                                wo_sb[p][:, 512 * hf:512 * (hf + 1)],
                                start=(p == 0), stop=(p == 3))
                        osb = op_.tile([128, 512], F32, name="osb", tag="osb",
                                       bufs=3)
                        nc.vector.tensor_copy(osb[:], ps[:])
                        nc.sync.dma_start(
                            out[128 * t8:128 * (t8 + 1),
                                512 * hf:512 * (hf + 1)], osb[:])

    split_multi_waits(nc)
    return nc


_CACHED_NC = None


def get_program():
    global _CACHED_NC
    if _CACHED_NC is None:
        _CACHED_NC = build_program()
    return _CACHED_NC


def make_in_maps(x, Wq, bq, Wk, bk, Wv, bv, Wo, bo):
    x = np.asarray(x, np.float32)
    bf = ml_dtypes.bfloat16
    WqT = np.ascontiguousarray(np.asarray(Wq, np.float32).T)
    WkT = np.ascontiguousarray(np.asarray(Wk, np.float32).T)
    WvT = np.ascontiguousarray(np.asarray(Wv, np.float32).T)
    WoT = np.ascontiguousarray(np.asarray(Wo, np.float32).T)
    bq = np.asarray(bq, np.float32)
    bk = np.asarray(bk, np.float32)
    bv = np.asarray(bv, np.float32)
    shard = []
    for hg in range(2):
        ds = slice(DC * hg, DC * (hg + 1))
        shard.append({
            "wqT": np.ascontiguousarray(WqT[:, ds]).astype(bf),
            "wkT": np.ascontiguousarray(WkT[:, ds]).astype(bf),
            "wvT": np.ascontiguousarray(WvT[:, ds]).astype(bf),
            "woT": np.ascontiguousarray(WoT[ds, :]).astype(bf),
            "bq2": np.ascontiguousarray(bq[ds].reshape(NP, 128).T),
            "bk2": np.ascontiguousarray(bk[ds].reshape(NP, 128).T),
            "bvb": np.ascontiguousarray(np.tile(bv[ds], (128, 1))),
            "ones4": np.ones((4, 64), np.float32),
            "tag": np.zeros((1, VARIANT), np.float32),
        })
    in_maps = []
    for c in range(NCORES):
        b, hg = c // 2, c % 2
        m = dict(shard[hg])
        m["xT"] = np.ascontiguousarray(x[b].T).astype(bf)
        in_maps.append(m)
    return in_maps


def assemble(results, bo):
    """Unshard: sum the two row-parallel partials per batch element + bias."""
    bo = np.asarray(bo, np.float32)
    out = np.empty((4, S, D), np.float32)
    for b in range(4):
        out[b] = results[2 * b]["out"] + results[2 * b + 1]["out"] + bo
    return out


def kernel(x, Wq, bq, Wk, bk, Wv, bv, Wo, bo):
    nc = get_program()
    in_maps = make_in_maps(x, Wq, bq, Wk, bk, Wv, bv, Wo, bo)
    res = run_bass_kernel_spmd(nc, in_maps, list(range(NCORES)))
    return assemble(res.results, bo)


# revision 41
# speedup vs baseline: 1.2695x; 1.0106x over previous
"""Multi-head attention (dense transformer block) on 8 Trainium2 NeuronCores.

Sharding: (batch=4) x (head-group=2) -> 8 shards, tensor-parallel over heads.
Core c handles batch b = c//2 and heads [8*hg, 8*hg+8) with hg = c%2:
Q/K/V weights column-sharded (512 of 1024 output dims per core), Wo
row-sharded; the two row-parallel partial outputs per batch element are
summed host-side (plus the bo bias) during the unshard. No collectives.

Per core: V projection into a per-head 65-column interleave (trailing ones
column fuses the softmax denominator into the attn.V matmul); per head pair
p (0..3): Q/K projected transposed [128, 2048]; per 512-query block, scores
for both heads land in one 2-bank PSUM tile via a row-paired matmul pair,
one wide exp(St/8) on ACT, and one M=65 matmul per head accumulates
[V|ones].T @ Pt over the 16 key chunks.  The numerators are evacuated to
ot[p] unnormalized (bf16), the two Z rows staged at partitions 0/32 of a
zr tile, one DVE reciprocal per query block, 1/Z broadcast down 64
partitions with ones outer-product matmuls, and one in-place multiply pair
normalizes ot.  Finally out_partial = Ot.T @ WoShard.T (f32, no bias).

Matmul operands are bf16 (fp32 PSUM accumulation).
"""

import numpy as np
import ml_dtypes

import concourse.bass as bass
import concourse.tile as tile
import concourse.mybir as mybir
from concourse.bass_utils import run_bass_kernel_spmd

F32 = mybir.dt.float32
F32R = mybir.dt.float32r
BF16 = mybir.dt.bfloat16
EXP = mybir.ActivationFunctionType.Exp

D = 1024          # d_model
S = 2048          # sequence length (full batch element per core)
NH = 16           # heads total
NHC = 8           # heads per core
NP = 4            # head pairs per core
DH = 64           # head dim
DC = 512          # output dims per core (NHC * DH)
NCORES = 8
VARIANT = 16      # bump to bust the HLO-signature-keyed NEFF cache


def split_multi_waits(nc):
    """The walrus build in this container accepts at most one sync-wait per
    instruction; move extra waits onto same-engine nops inserted before the
    offending instruction."""
    k = 0
    for f in nc.m.functions:
        for bb in f.blocks:
            out, changed = [], False
            for inst in bb.instructions:
                si = inst.sync_info
                waits = list(si.on_wait) if si and si.on_wait else []
                if len(waits) > 1:
                    changed = True
                    for w in waits[:-1]:
                        nop = mybir.InstNoOp(name=f"wsplit-{k}", ins=[], outs=[])
                        k += 1
                        nop.engine = inst.engine
                        nop.sync_info = mybir.SyncInfo(on_wait=[w], on_update=[])
                        nc.register_instruction(nop, overwrite=True)
                        out.append(nop)
                    si.on_wait = waits[-1:]
                out.append(inst)
            if changed:
                bb.instructions = out


def build_program():
    nc = bass.Bass()
    xT = nc.declare_dram_parameter("xT", [D, S], BF16, isOutput=False)
    wkT = nc.declare_dram_parameter("wkT", [D, DC], BF16, isOutput=False)
    wqT = nc.declare_dram_parameter("wqT", [D, DC], BF16, isOutput=False)
    wvT = nc.declare_dram_parameter("wvT", [D, DC], BF16, isOutput=False)
    bk2 = nc.declare_dram_parameter("bk2", [128, NP], F32, isOutput=False)
    bq2 = nc.declare_dram_parameter("bq2", [128, NP], F32, isOutput=False)
    bvb = nc.declare_dram_parameter("bvb", [128, DC], F32, isOutput=False)
    ones4 = nc.declare_dram_parameter("ones4", [4, 64], F32R, isOutput=False)
    tag = nc.declare_dram_parameter("tag", [1, VARIANT], F32, isOutput=False)
    woT = nc.declare_dram_parameter("woT", [DC, D], BF16, isOutput=False)
    out = nc.declare_dram_parameter("out", [S, D], F32, isOutput=True)

    with tile.TileContext(nc) as tc:
        with tc.tile_pool(name="pp", bufs=1) as pp, \
             tc.tile_pool(name="qk", bufs=2) as qkp, \
             tc.tile_pool(name="pt", bufs=3) as ptp, \
             tc.tile_pool(name="zr", bufs=2) as zrp, \
             tc.tile_pool(name="psS", bufs=2, space="PSUM") as stp, \
             tc.tile_pool(name="psA", bufs=4, space="PSUM") as psa:
            bk_sb = pp.tile([128, 16], F32, name="bk_sb", tag="bk_sb")
            bq_sb = pp.tile([128, 16], F32, name="bq_sb", tag="bq_sb")
            bvb_sb = pp.tile([128, DC], F32, name="bvb_sb", tag="bvb_sb")
            ones_sb = pp.tile([128, 64], F32R, name="ones_sb", tag="ones_sb")
            tag_sb = pp.tile([1, VARIANT], F32, name="tag_sb", tag="tag_sb")
            nc.sync.dma_start(bk_sb[:, 0:NP], bk2[:])
            nc.sync.dma_start(bq_sb[:, 0:NP], bq2[:])
            nc.sync.dma_start(bvb_sb[:], bvb[:])
            nc.sync.dma_start(tag_sb[:], tag[:])
            for s in range(4):
                nc.sync.dma_start(ones_sb[32 * s:32 * s + 1, :], ones4[s:s + 1, :])
            xt_sb = [pp.tile([128, S], BF16, name=f"xt{d}", tag=f"xt{d}")
                     for d in range(8)]
            wk_sb = [pp.tile([128, DC], BF16, name=f"wk{d}", tag=f"wk{d}")
                     for d in range(8)]
            wq_sb = [pp.tile([128, DC], BF16, name=f"wq{d}", tag=f"wq{d}")
                     for d in range(8)]
            wv_sb = [pp.tile([128, DC], BF16, name=f"wv{d}", tag=f"wv{d}")
                     for d in range(8)]
            for d in range(8):
                nc.sync.dma_start(wk_sb[d][:], wkT[128 * d:128 * (d + 1), :])
            for d in range(8):
                nc.sync.dma_start(wq_sb[d][:], wqT[128 * d:128 * (d + 1), :])
            for c in range(4):
                cs = slice(512 * c, 512 * (c + 1))
                for d in range(8):
                    nc.sync.dma_start(xt_sb[d][:, cs], xT[128 * d:128 * (d + 1), cs])
            for d in range(8):
                nc.sync.dma_start(wv_sb[d][:], wvT[128 * d:128 * (d + 1), :])
            vg = [pp.tile([128, 520], BF16, name=f"vg{t}", tag=f"vg{t}")
                  for t in range(16)]
            ots = [pp.tile([128, S], BF16, name=f"ot{i}", tag=f"ot{i}")
                   for i in range(NP)]

            def qk_proj(p):
                kt_p = qkp.tile([128, S], BF16, name="kt_p", tag="kt", bufs=2)
                qt_p = qkp.tile([128, S], BF16, name="qt_p", tag="qt", bufs=2)
                # interleave K and Q per token block so each xt chunk feeds
                # both projections as soon as it lands (halves time-to-first-
                # score at kernel start)
                for tb in range(4):
                    ts = slice(512 * tb, 512 * (tb + 1))
                    ps = psa.tile([128, 512], F32, name="psk", tag="psa", bufs=4)
                    for d in range(8):
                        nc.tensor.matmul(
                            ps[:], wk_sb[d][:, 128 * p:128 * (p + 1)],
                            xt_sb[d][:, ts], start=(d == 0), stop=(d == 7))
                    nc.vector.tensor_scalar_add(kt_p[:, ts], ps[:],
                                                bk_sb[:, p:p + 1])
                    ps = psa.tile([128, 512], F32, name="psq", tag="psa", bufs=4)
                    for d in range(8):
                        nc.tensor.matmul(
                            ps[:], wq_sb[d][:, 128 * p:128 * (p + 1)],
                            xt_sb[d][:, ts], start=(d == 0), stop=(d == 7))
                    nc.vector.tensor_scalar_add(qt_p[:, ts], ps[:],
                                                bq_sb[:, p:p + 1])
                return kt_p, qt_p

            def make_tail(p, qs2, zr):
                """1/Z normalization for one finished query block.  Emitted
                one block LATE so the PE's in-order stream never stalls on
                the broadcast matmuls waiting for the DVE reciprocal."""
                def tail():
                    rz = zrp.tile([128, 512], F32R, name="rz", tag="rz", bufs=2)
                    with nc.allow_low_precision(reason="1/Z to f32r matmul"):
                        nc.vector.reciprocal(rz[0:33, :], zr[0:33, :])
                    pb0 = psa.tile([128, 512], F32, name="pb0", tag="psa", bufs=4)
                    pb1 = psa.tile([128, 512], F32, name="pb1", tag="psa", bufs=4)
                    nc.tensor.matmul(pb0[0:64, :], ones_sb[0:1, :], rz[0:1, :],
                                     start=True, stop=True)
                    nc.tensor.matmul(pb1[0:64, :], ones_sb[32:33, :],
                                     rz[32:33, :], start=True, stop=True)
                    nc.vector.tensor_mul(ots[p][0:64, qs2], ots[p][0:64, qs2],
                                         pb0[0:64, :])
                    nc.vector.tensor_mul(ots[p][64:128, qs2],
                                         ots[p][64:128, qs2], pb1[0:64, :])
                return tail

            pending = None
            kt_p, qt_p = qk_proj(0)
            for p in range(NP):
                if p > 0:
                    kt_p, qt_p = qk_proj(p)
                c0, c1 = 130 * p, 130 * p + 65
                for qb in range(4):
                    qs2 = slice(512 * qb, 512 * (qb + 1))
                    po0 = psa.tile([128, 512], F32, name="po0", tag="psa", bufs=4)
                    po1 = psa.tile([128, 512], F32, name="po1", tag="psa", bufs=4)
                    for k in range(16):
                        if p == 0 and qb == 0:
                            # fused V projection for key chunk k
                            ps = psa.tile([128, 512], F32, name="psv",
                                          tag="psa", bufs=4)
                            for d in range(8):
                                nc.tensor.matmul(
                                    ps[:], xt_sb[d][:, 128 * k:128 * (k + 1)],
                                    wv_sb[d][:], start=(d == 0), stop=(d == 7))
                            dst = vg[k][:].rearrange(
                                "p (h w) -> p h w", w=65)[:, :, 0:64]
                            nc.vector.tensor_add(
                                dst, ps[:].rearrange("p (h w) -> p h w", w=64),
                                bvb_sb[:].rearrange("p (h w) -> p h w", w=64))
                            nc.vector.memset(
                                vg[k][:].rearrange(
                                    "p (h w) -> p h w", w=65)[:, :, 64:65], 1.0)
                        ks = slice(128 * k, 128 * (k + 1))
                        st = stp.tile([128, 1024], F32, name="st", tag="st",
                                      bufs=2)
                        nc.tensor.matmul(st[:, 0:512], kt_p[0:64, ks],
                                         qt_p[0:64, qs2], start=True, stop=True)
                        nc.tensor.matmul(st[:, 512:1024], kt_p[64:128, ks],
                                         qt_p[64:128, qs2], start=True, stop=True)
                        pt = ptp.tile([128, 1024], BF16, name="pt", tag="pt",
                                      bufs=3)
                        nc.scalar.activation(pt[:], st[:], EXP, scale=0.125)
                        first, last = (k == 0), (k == 15)
                        nc.tensor.matmul(po0[0:65, :], vg[k][:, c0:c0 + 65],
                                         pt[:, 0:512], start=first, stop=last)
                        nc.tensor.matmul(po1[0:65, :], vg[k][:, c1:c1 + 65],
                                         pt[:, 512:1024], start=first, stop=last)
                    zr = zrp.tile([128, 512], F32, name="zrt", tag="zrt", bufs=2)
                    nc.vector.tensor_copy(zr[0:1, :], po0[64:65, :])
                    nc.vector.tensor_copy(zr[32:33, :], po1[64:65, :])
                    # evacuate numerators on the (idle) scalar engine so the
                    # po slots free while DVE handles the Z rows
                    nc.scalar.copy(ots[p][0:64, qs2], po0[0:64, :])
                    nc.scalar.copy(ots[p][64:128, qs2], po1[0:64, :])
                    if pending is not None:
                        pending()
                    pending = make_tail(p, qs2, zr)
            # the final tail (p3, tokens 1536:2048) only feeds O-proj token
            # blocks 12-15; defer it past the first 12 so its reciprocal
            # overlaps the O-projection matmul stream instead of stalling it

            with tc.tile_pool(name="wo", bufs=1) as wop, \
                 tc.tile_pool(name="osb", bufs=3) as op_:
                wo_sb = [wop.tile([128, 1024], BF16, name=f"wo{d}", tag=f"wo{d}")
                         for d in range(4)]
                for d in range(4):
                    nc.sync.dma_start(wo_sb[d][:], woT[128 * d:128 * (d + 1), :])
                for t8 in range(16):
                    if t8 == 12:
                        pending()
                        pending = None
                    for hf in range(2):
                        ps = psa.tile([128, 512], F32, name="pso", tag="psa",
                                      bufs=4)
                        for p in range(4):
                            nc.tensor.matmul(
                                ps[:], ots[p][:, 128 * t8:128 * (t8 + 1)],
                                wo_sb[p][:, 512 * hf:512 * (hf + 1)],
                                start=(p == 0), stop=(p == 3))
                        osb = op_.tile([128, 512], F32, name="osb", tag="osb",
                                       bufs=3)
                        nc.vector.tensor_copy(osb[:], ps[:])
                        nc.sync.dma_start(
                            out[128 * t8:128 * (t8 + 1),
                                512 * hf:512 * (hf + 1)], osb[:])

    split_multi_waits(nc)
    return nc


_CACHED_NC = None


def get_program():
    global _CACHED_NC
    if _CACHED_NC is None:
        _CACHED_NC = build_program()
    return _CACHED_NC


def make_in_maps(x, Wq, bq, Wk, bk, Wv, bv, Wo, bo):
    x = np.asarray(x, np.float32)
    bf = ml_dtypes.bfloat16
    WqT = np.ascontiguousarray(np.asarray(Wq, np.float32).T)
    WkT = np.ascontiguousarray(np.asarray(Wk, np.float32).T)
    WvT = np.ascontiguousarray(np.asarray(Wv, np.float32).T)
    WoT = np.ascontiguousarray(np.asarray(Wo, np.float32).T)
    bq = np.asarray(bq, np.float32)
    bk = np.asarray(bk, np.float32)
    bv = np.asarray(bv, np.float32)
    shard = []
    for hg in range(2):
        ds = slice(DC * hg, DC * (hg + 1))
        shard.append({
            "wqT": np.ascontiguousarray(WqT[:, ds]).astype(bf),
            "wkT": np.ascontiguousarray(WkT[:, ds]).astype(bf),
            "wvT": np.ascontiguousarray(WvT[:, ds]).astype(bf),
            "woT": np.ascontiguousarray(WoT[ds, :]).astype(bf),
            "bq2": np.ascontiguousarray(bq[ds].reshape(NP, 128).T),
            "bk2": np.ascontiguousarray(bk[ds].reshape(NP, 128).T),
            "bvb": np.ascontiguousarray(np.tile(bv[ds], (128, 1))),
            "ones4": np.ones((4, 64), np.float32),
            "tag": np.zeros((1, VARIANT), np.float32),
        })
    in_maps = []
    for c in range(NCORES):
        b, hg = c // 2, c % 2
        m = dict(shard[hg])
        m["xT"] = np.ascontiguousarray(x[b].T).astype(bf)
        in_maps.append(m)
    return in_maps


def assemble(results, bo):
    """Unshard: sum the two row-parallel partials per batch element + bias."""
    bo = np.asarray(bo, np.float32)
    out = np.empty((4, S, D), np.float32)
    for b in range(4):
        out[b] = results[2 * b]["out"] + results[2 * b + 1]["out"] + bo
    return out


def kernel(x, Wq, bq, Wk, bk, Wv, bv, Wo, bo):
    nc = get_program()
    in_maps = make_in_maps(x, Wq, bq, Wk, bk, Wv, bv, Wo, bo)
    res = run_bass_kernel_spmd(nc, in_maps, list(range(NCORES)))
    return assemble(res.results, bo)


# revision 46
# speedup vs baseline: 1.3974x; 1.1008x over previous
"""Multi-head attention (dense transformer block) on 8 Trainium2 NeuronCores.

Sharding: (batch=4) x (head-group=2) -> 8 shards, tensor-parallel over heads.
Core c handles batch b = c//2 and heads [8*hg, 8*hg+8) with hg = c%2:
Q/K/V weights column-sharded (512 of 1024 output dims per core), Wo
row-sharded; the two row-parallel partial outputs per batch element are
summed host-side (plus the bo bias) during the unshard. No collectives.

Per core: V projection into a per-head 65-column interleave (trailing ones
column fuses the softmax denominator into the attn.V matmul); per head pair
p (0..3): Q/K projected transposed [128, 2048]; per 512-query block, scores
for both heads land in one 2-bank PSUM tile via a row-paired matmul pair,
one wide exp(St/8) on ACT, and one M=65 matmul per head accumulates
[V|ones].T @ Pt over the 16 key chunks.  The numerators are evacuated to
ot[p] unnormalized (bf16), the two Z rows staged at partitions 0/32 of a
zr tile, one DVE reciprocal per query block, 1/Z broadcast down 64
partitions with ones outer-product matmuls, and one in-place multiply pair
normalizes ot.  Finally out_partial = Ot.T @ WoShard.T (f32, no bias).

Matmul operands are bf16 (fp32 PSUM accumulation).
"""

import numpy as np
import ml_dtypes

import concourse.bass as bass
import concourse.tile as tile
import concourse.mybir as mybir
from concourse.bass_utils import run_bass_kernel_spmd

F32 = mybir.dt.float32
F32R = mybir.dt.float32r
BF16 = mybir.dt.bfloat16
EXP = mybir.ActivationFunctionType.Exp

D = 1024          # d_model
S = 2048          # sequence length (full batch element per core)
NH = 16           # heads total
NHC = 8           # heads per core
NP = 4            # head pairs per core
DH = 64           # head dim
DC = 512          # output dims per core (NHC * DH)
NCORES = 8
VARIANT = 17      # bump to bust the HLO-signature-keyed NEFF cache


def split_multi_waits(nc):
    """The walrus build in this container accepts at most one sync-wait per
    instruction; move extra waits onto same-engine nops inserted before the
    offending instruction."""
    k = 0
    for f in nc.m.functions:
        for bb in f.blocks:
            out, changed = [], False
            for inst in bb.instructions:
                si = inst.sync_info
                waits = list(si.on_wait) if si and si.on_wait else []
                if len(waits) > 1:
                    changed = True
                    for w in waits[:-1]:
                        nop = mybir.InstNoOp(name=f"wsplit-{k}", ins=[], outs=[])
                        k += 1
                        nop.engine = inst.engine
                        nop.sync_info = mybir.SyncInfo(on_wait=[w], on_update=[])
                        nc.register_instruction(nop, overwrite=True)
                        out.append(nop)
                    si.on_wait = waits[-1:]
                out.append(inst)
            if changed:
                bb.instructions = out


def build_program():
    nc = bass.Bass()
    xT = nc.declare_dram_parameter("xT", [D, S], BF16, isOutput=False)
    wkT = nc.declare_dram_parameter("wkT", [D, DC], BF16, isOutput=False)
    wqT = nc.declare_dram_parameter("wqT", [D, DC], BF16, isOutput=False)
    wvT = nc.declare_dram_parameter("wvT", [D, DC], BF16, isOutput=False)
    bk2 = nc.declare_dram_parameter("bk2", [128, NP], F32, isOutput=False)
    bq2 = nc.declare_dram_parameter("bq2", [128, NP], F32, isOutput=False)
    bvb = nc.declare_dram_parameter("bvb", [128, DC], F32, isOutput=False)
    ones4 = nc.declare_dram_parameter("ones4", [4, 64], F32R, isOutput=False)
    tag = nc.declare_dram_parameter("tag", [1, VARIANT], F32, isOutput=False)
    woT = nc.declare_dram_parameter("woT", [DC, D], BF16, isOutput=False)
    out = nc.declare_dram_parameter("out", [S, D], F32, isOutput=True)

    with tile.TileContext(nc) as tc:
        with tc.tile_pool(name="pp", bufs=1) as pp, \
             tc.tile_pool(name="qk", bufs=2) as qkp, \
             tc.tile_pool(name="pt", bufs=3) as ptp, \
             tc.tile_pool(name="zr", bufs=2) as zrp, \
             tc.tile_pool(name="psS", bufs=2, space="PSUM") as stp, \
             tc.tile_pool(name="psA", bufs=4, space="PSUM") as psa:
            bk_sb = pp.tile([128, 16], F32, name="bk_sb", tag="bk_sb")
            bq_sb = pp.tile([128, 16], F32, name="bq_sb", tag="bq_sb")
            bvb_sb = pp.tile([128, DC], F32, name="bvb_sb", tag="bvb_sb")
            ones_sb = pp.tile([128, 64], F32R, name="ones_sb", tag="ones_sb")
            tag_sb = pp.tile([1, VARIANT], F32, name="tag_sb", tag="tag_sb")
            nc.sync.dma_start(bk_sb[:, 0:NP], bk2[:])
            nc.sync.dma_start(bq_sb[:, 0:NP], bq2[:])
            nc.sync.dma_start(bvb_sb[:], bvb[:])
            nc.sync.dma_start(tag_sb[:], tag[:])
            for s in range(4):
                nc.sync.dma_start(ones_sb[32 * s:32 * s + 1, :], ones4[s:s + 1, :])
            xt_sb = [pp.tile([128, S], BF16, name=f"xt{d}", tag=f"xt{d}")
                     for d in range(8)]
            wk_sb = [pp.tile([128, DC], BF16, name=f"wk{d}", tag=f"wk{d}")
                     for d in range(8)]
            wq_sb = [pp.tile([128, DC], BF16, name=f"wq{d}", tag=f"wq{d}")
                     for d in range(8)]
            wv_sb = [pp.tile([128, DC], BF16, name=f"wv{d}", tag=f"wv{d}")
                     for d in range(8)]
            for d in range(8):
                nc.sync.dma_start(wk_sb[d][:], wkT[128 * d:128 * (d + 1), :])
            for d in range(8):
                nc.sync.dma_start(wq_sb[d][:], wqT[128 * d:128 * (d + 1), :])
            for c in range(4):
                cs = slice(512 * c, 512 * (c + 1))
                for d in range(8):
                    nc.sync.dma_start(xt_sb[d][:, cs], xT[128 * d:128 * (d + 1), cs])
            for d in range(8):
                nc.sync.dma_start(wv_sb[d][:], wvT[128 * d:128 * (d + 1), :])
            vg = [pp.tile([128, 520], BF16, name=f"vg{t}", tag=f"vg{t}")
                  for t in range(16)]
            ots = [pp.tile([128, S], BF16, name=f"ot{i}", tag=f"ot{i}")
                   for i in range(NP)]

            def qk_tiles():
                kt_p = qkp.tile([128, S], BF16, name="kt_p", tag="kt", bufs=2)
                qt_p = qkp.tile([128, S], BF16, name="qt_p", tag="qt", bufs=2)
                return kt_p, qt_p

            def qk_group(p, tb, kt_p, qt_p):
                """K+Q projection for one 512-token block of head pair p."""
                ts = slice(512 * tb, 512 * (tb + 1))
                ps = psa.tile([128, 512], F32, name="psk", tag="psa", bufs=4)
                for d in range(8):
                    nc.tensor.matmul(
                        ps[:], wk_sb[d][:, 128 * p:128 * (p + 1)],
                        xt_sb[d][:, ts], start=(d == 0), stop=(d == 7))
                nc.vector.tensor_scalar_add(kt_p[:, ts], ps[:],
                                            bk_sb[:, p:p + 1])
                ps = psa.tile([128, 512], F32, name="psq", tag="psa", bufs=4)
                for d in range(8):
                    nc.tensor.matmul(
                        ps[:], wq_sb[d][:, 128 * p:128 * (p + 1)],
                        xt_sb[d][:, ts], start=(d == 0), stop=(d == 7))
                nc.vector.tensor_scalar_add(qt_p[:, ts], ps[:],
                                            bq_sb[:, p:p + 1])

            def make_tail(p, qs2, zr):
                """1/Z normalization for one finished query block.  Emitted
                one block LATE so the PE's in-order stream never stalls on
                the broadcast matmuls waiting for the DVE reciprocal."""
                def tail():
                    rz = zrp.tile([128, 512], F32R, name="rz", tag="rz", bufs=2)
                    with nc.allow_low_precision(reason="1/Z to f32r matmul"):
                        nc.vector.reciprocal(rz[0:33, :], zr[0:33, :])
                    pb0 = psa.tile([128, 512], F32, name="pb0", tag="psa", bufs=4)
                    pb1 = psa.tile([128, 512], F32, name="pb1", tag="psa", bufs=4)
                    nc.tensor.matmul(pb0[0:64, :], ones_sb[0:1, :], rz[0:1, :],
                                     start=True, stop=True)
                    nc.tensor.matmul(pb1[0:64, :], ones_sb[32:33, :],
                                     rz[32:33, :], start=True, stop=True)
                    nc.vector.tensor_mul(ots[p][0:64, qs2], ots[p][0:64, qs2],
                                         pb0[0:64, :])
                    nc.vector.tensor_mul(ots[p][64:128, qs2],
                                         ots[p][64:128, qs2], pb1[0:64, :])
                return tail

            pending = None
            cur = qk_tiles()
            for tb in range(4):
                qk_group(0, tb, *cur)
            for p in range(NP):
                kt_p, qt_p = cur
                # next pair's projection groups are woven in one per query
                # block below, filling the PE bubble at each block boundary
                nxt = qk_tiles() if p + 1 < NP else None
                c0, c1 = 130 * p, 130 * p + 65
                for qb in range(4):
                    qs2 = slice(512 * qb, 512 * (qb + 1))
                    po0 = psa.tile([128, 512], F32, name="po0", tag="psa", bufs=4)
                    po1 = psa.tile([128, 512], F32, name="po1", tag="psa", bufs=4)
                    for k in range(16):
                        if p == 0 and qb == 0:
                            # fused V projection for key chunk k
                            ps = psa.tile([128, 512], F32, name="psv",
                                          tag="psa", bufs=4)
                            for d in range(8):
                                nc.tensor.matmul(
                                    ps[:], xt_sb[d][:, 128 * k:128 * (k + 1)],
                                    wv_sb[d][:], start=(d == 0), stop=(d == 7))
                            dst = vg[k][:].rearrange(
                                "p (h w) -> p h w", w=65)[:, :, 0:64]
                            nc.vector.tensor_add(
                                dst, ps[:].rearrange("p (h w) -> p h w", w=64),
                                bvb_sb[:].rearrange("p (h w) -> p h w", w=64))
                            nc.vector.memset(
                                vg[k][:].rearrange(
                                    "p (h w) -> p h w", w=65)[:, :, 64:65], 1.0)
                        ks = slice(128 * k, 128 * (k + 1))
                        st = stp.tile([128, 1024], F32, name="st", tag="st",
                                      bufs=2)
                        nc.tensor.matmul(st[:, 0:512], kt_p[0:64, ks],
                                         qt_p[0:64, qs2], start=True, stop=True)
                        nc.tensor.matmul(st[:, 512:1024], kt_p[64:128, ks],
                                         qt_p[64:128, qs2], start=True, stop=True)
                        pt = ptp.tile([128, 1024], BF16, name="pt", tag="pt",
                                      bufs=3)
                        nc.scalar.activation(pt[:], st[:], EXP, scale=0.125)
                        first, last = (k == 0), (k == 15)
                        nc.tensor.matmul(po0[0:65, :], vg[k][:, c0:c0 + 65],
                                         pt[:, 0:512], start=first, stop=last)
                        nc.tensor.matmul(po1[0:65, :], vg[k][:, c1:c1 + 65],
                                         pt[:, 512:1024], start=first, stop=last)
                    zr = zrp.tile([128, 512], F32, name="zrt", tag="zrt", bufs=2)
                    nc.vector.tensor_copy(zr[0:1, :], po0[64:65, :])
                    nc.vector.tensor_copy(zr[32:33, :], po1[64:65, :])
                    # evacuate numerators on the (idle) scalar engine so the
                    # po slots free while DVE handles the Z rows
                    nc.scalar.copy(ots[p][0:64, qs2], po0[0:64, :])
                    nc.scalar.copy(ots[p][64:128, qs2], po1[0:64, :])
                    if nxt is not None:
                        qk_group(p + 1, qb, *nxt)
                    if pending is not None:
                        pending()
                    pending = make_tail(p, qs2, zr)
                cur = nxt
            # the final tail (p3, tokens 1536:2048) only feeds O-proj token
            # blocks 12-15; defer it past the first 12 so its reciprocal
            # overlaps the O-projection matmul stream instead of stalling it

            with tc.tile_pool(name="wo", bufs=1) as wop, \
                 tc.tile_pool(name="osb", bufs=3) as op_:
                wo_sb = [wop.tile([128, 1024], BF16, name=f"wo{d}", tag=f"wo{d}")
                         for d in range(4)]
                for d in range(4):
                    nc.sync.dma_start(wo_sb[d][:], woT[128 * d:128 * (d + 1), :])
                for t8 in range(16):
                    if t8 == 12:
                        pending()
                        pending = None
                    for hf in range(2):
                        ps = psa.tile([128, 512], F32, name="pso", tag="psa",
                                      bufs=4)
                        for p in range(4):
                            nc.tensor.matmul(
                                ps[:], ots[p][:, 128 * t8:128 * (t8 + 1)],
                                wo_sb[p][:, 512 * hf:512 * (hf + 1)],
                                start=(p == 0), stop=(p == 3))
                        osb = op_.tile([128, 512], F32, name="osb", tag="osb",
                                       bufs=3)
                        nc.vector.tensor_copy(osb[:], ps[:])
                        nc.sync.dma_start(
                            out[128 * t8:128 * (t8 + 1),
                                512 * hf:512 * (hf + 1)], osb[:])

    split_multi_waits(nc)
    return nc


_CACHED_NC = None


def get_program():
    global _CACHED_NC
    if _CACHED_NC is None:
        _CACHED_NC = build_program()
    return _CACHED_NC


def make_in_maps(x, Wq, bq, Wk, bk, Wv, bv, Wo, bo):
    x = np.asarray(x, np.float32)
    bf = ml_dtypes.bfloat16
    WqT = np.ascontiguousarray(np.asarray(Wq, np.float32).T)
    WkT = np.ascontiguousarray(np.asarray(Wk, np.float32).T)
    WvT = np.ascontiguousarray(np.asarray(Wv, np.float32).T)
    WoT = np.ascontiguousarray(np.asarray(Wo, np.float32).T)
    bq = np.asarray(bq, np.float32)
    bk = np.asarray(bk, np.float32)
    bv = np.asarray(bv, np.float32)
    shard = []
    for hg in range(2):
        ds = slice(DC * hg, DC * (hg + 1))
        shard.append({
            "wqT": np.ascontiguousarray(WqT[:, ds]).astype(bf),
            "wkT": np.ascontiguousarray(WkT[:, ds]).astype(bf),
            "wvT": np.ascontiguousarray(WvT[:, ds]).astype(bf),
            "woT": np.ascontiguousarray(WoT[ds, :]).astype(bf),
            "bq2": np.ascontiguousarray(bq[ds].reshape(NP, 128).T),
            "bk2": np.ascontiguousarray(bk[ds].reshape(NP, 128).T),
            "bvb": np.ascontiguousarray(np.tile(bv[ds], (128, 1))),
            "ones4": np.ones((4, 64), np.float32),
            "tag": np.zeros((1, VARIANT), np.float32),
        })
    in_maps = []
    for c in range(NCORES):
        b, hg = c // 2, c % 2
        m = dict(shard[hg])
        m["xT"] = np.ascontiguousarray(x[b].T).astype(bf)
        in_maps.append(m)
    return in_maps


def assemble(results, bo):
    """Unshard: sum the two row-parallel partials per batch element + bias."""
    bo = np.asarray(bo, np.float32)
    out = np.empty((4, S, D), np.float32)
    for b in range(4):
        out[b] = results[2 * b]["out"] + results[2 * b + 1]["out"] + bo
    return out


def kernel(x, Wq, bq, Wk, bk, Wv, bv, Wo, bo):
    nc = get_program()
    in_maps = make_in_maps(x, Wq, bq, Wk, bk, Wv, bv, Wo, bo)
    res = run_bass_kernel_spmd(nc, in_maps, list(range(NCORES)))
    return assemble(res.results, bo)
